# revision 6
# baseline (speedup 1.0000x reference)
"""Trainium2 Bass kernel v2 for the GAT-style attention head (B=2, N=6144, H=256, O=128).

Math (matching the reference):
  seq_fts = seq @ W_fts.T                       [B, N, O]
  f1 = seq_fts @ f1_w + f1_b                    [B, N]
  f2 = seq_fts @ f2_w + f2_b                    [B, N]
  z[b, j, i]  = leaky_relu(f1[b, i] + f2[b, j], 0.01)
  coefs[b,j,i] = softmax_b(z)   (B=2: c0 = sigmoid(z0 - z1), c1 = 1 - c0)
  vals[b, i, o] = sum_j coefs[b, j, i] * seq_fts[b, j, o]
  out = elu(vals + bias)

Key reformulation: c0 - 0.5 = 0.5*tanh((z0-z1)/2), and leaky_relu is
positively homogeneous, so with HALVED f1/f2 (0.5 folded into the host-packed
weights) the device computes
  dh[j,i] = lrelu(f1h[i]+f2h[j])|b0 - lrelu(...)|b1     (fused custom DVE op)
  tc = tanh(dh)                                          (ACT, fp8 out)
  P[b]  = sum_j tc[j,i] * fts_half[b,j,o]                (fp8 DoubleRow matmuls,
                                                          transposed acc [o, i])
  vals[0] = Sh[0] + P[0],  vals[1] = Sh[1] - P[1]        (Sh = colsum fts_half)
  out = elu(vals + bias) = relu(y) - 1 + exp(min(y, 0))

Sharding: each core owns 768 output rows i; seq streams fully through every
core (j loop). Host prep: seq pre-transposed/cast to bf16 [2b,2hc,128h,N],
f1h (own shard) / f2h (all j) computed on host fp32, W^T*0.5 packed bf16.
Output written transposed [B, O, NS] and fixed up on the host.

Schedule: 12 blocks of 2 j-tile-pairs (512 j rows each). Steady state is
DVE-bound (4 fused lrelu-diff ops per block); psum->sbuf fts casts ride
mostly on ACT (cfg dve_copy_every), tanh is one ACT op per block.
"""

import numpy as np
import ml_dtypes

import concourse.bacc as bacc
import concourse.bass as bass
import concourse.mybir as mybir
import concourse.tile as tile
from concourse.bass_utils import run_bass_kernel_spmd

B, N, H, O = 2, 6144, 256, 128
NCORES = 8
NS = N // NCORES          # 768 i-rows per core
NBLK = N // 512           # 12 blocks, each 2 pairs = 4 j-tiles = 512 j rows
FP32 = mybir.dt.float32
BF16 = mybir.dt.bfloat16
FP8 = mybir.dt.float8e4
AF = mybir.ActivationFunctionType
ALU = mybir.AluOpType
DR = mybir.MatmulPerfMode.DoubleRow

_DVE_OP_NAME = "DIFF_LRELU_ANT"

DEFAULT_CFG = dict(
    lag=1,                # stage-B lags scores by this many blocks
    bufs_sT=3,
    bufs_fppD=2,
    bufs_f8=3,
    bufs_d=3,
    bufs_tc=3,
    stageb="dr8",         # dr8 | bf16
    dve_copy_blocks=(9, 10, 11),  # blocks whose fts cast runs on DVE (rest ACT)
    split_tanh_block=-1,  # block whose tanh is emitted per-subtile
    store_queues=1,
    fin_act_units=0,
    pe_warmup=16,
    fin_wide=False,
    split_load0=True,
    swap_last_b=True,
    fin_nm_act=False,
    f1bc_bf16=True,
    dma_order=0,
    fin_dve_units=3,      # finalize units (of 6) using the DVE ry/m path
)


def _get_diff_lrelu_op():
    """Register (once) and return the fused custom DVE op:
    out = lrelu(in0 + s0) - lrelu(in1 + s1), slope imm2."""
    import concourse.dve_ops as dve_ops
    from concourse.dve_ops import OPS, DveOp

    for op in OPS:
        if op.name == _DVE_OP_NAME:
            return op

    from concourse.dve_spec import C0, C1, C2, Spec, Src0, Src1, lower, maxx
    from concourse.dve_uop import DveOpSpec

    a = Src0 + C0
    b = Src1 + C1
    spec = Spec(
        body=maxx(a, a * C2) - maxx(b, b * C2),
        reference=lambda in0, in1, s0, s1, imm2: (
            np.maximum(in0 + s0, (in0 + s0) * imm2)
            - np.maximum(in1 + s1, (in1 + s1) * imm2)
        ).astype(np.float32),
    )
    row = dve_ops._CUSTOM_DVE_ROW_BASE + len(OPS)
    shas = {}
    for ver in ("v3",):
        uops = lower(spec, ver=ver)
        shas[ver] = DveOpSpec(
            name=_DVE_OP_NAME, opcode=row, uops=uops, rd1_en=True
        ).sha(ver)
    op = DveOp(_DVE_OP_NAME, spec, subdim=False, uops_sha=shas)
    OPS.append(op)
    dve_ops.CUSTOM_DVE_SPECS[_DVE_OP_NAME] = spec
    dve_ops._SUB_OPCODE_FOR_NAME[_DVE_OP_NAME] = row
    return op


def build_nc(cfg=None):
    cfg = {**DEFAULT_CFG, **(cfg or {})}
    diff_lrelu = _get_diff_lrelu_op()
    dr8 = cfg["stageb"] == "dr8"
    SBD = FP8 if dr8 else BF16     # stage-B operand dtype
    DVE_COPY_BLOCKS = set(cfg["dve_copy_blocks"])

    nc = bacc.Bacc("TRN2", target_bir_lowering=False, debug=False, num_devices=NCORES)

    # seqT: [2b, 2hc, 128h, N] bf16 (host-transposed)
    seqT_d = nc.declare_dram_parameter("seqT", [B, 2, 128, N], BF16, isOutput=False)
    # wtg: [2b, 128h, 2hc, 128o] bf16 = +/-0.5 * W^T chunks (b=1 negated so
    # the accumulator holds -P1 and finalize is uniform y = P' + s)
    wtg_d = nc.declare_dram_parameter("wtg", [B, 128, 2, O], BF16, isOutput=False)
    # f2h columns for all j: [128j, 2b, 48jt] fp32
    f2c_d = nc.declare_dram_parameter("f2c", [128, B, N // 128], FP32, isOutput=False)
    # f1h broadcast for own shard (bias folded in): [128, 2b*768]
    F1DT = BF16 if cfg["f1bc_bf16"] else FP32
    f1r_d = nc.declare_dram_parameter("f1r", [128, B * NS], F1DT, isOutput=False)
    # consts: [bias, bias-1, 0, 0]
    consts_d = nc.declare_dram_parameter("consts", [1, 4], FP32, isOutput=False)
    # transposed output [B, O, NS]
    out_d = nc.declare_dram_parameter("out", [B, O, NS], FP32, isOutput=True)

    NCH = 3                      # accumulator column chunks (2 per psum bank)
    CW = NS // NCH               # 256

    with tile.TileContext(nc) as tc:
        with (
            tc.tile_pool(name="const", bufs=1) as cpool,
            tc.tile_pool(name="sT", bufs=cfg["bufs_sT"]) as p_sT,
            tc.tile_pool(name="fts8", bufs=cfg["bufs_f8"]) as p_f8,
            tc.tile_pool(name="dt", bufs=cfg["bufs_d"]) as p_d,
            tc.tile_pool(name="tct", bufs=cfg["bufs_tc"]) as p_tc,
            tc.tile_pool(name="fin", bufs=14) as p_fin,
            tc.tile_pool(name="finw", bufs=1) as p_finw,
        ):
            # ---------------- constants / setup ----------------
            wtg = cpool.tile([128, B, 2, O], BF16)
            f1bc2 = cpool.tile([128, B * NS], F1DT)
            f2c = cpool.tile([128, B, N // 128], FP32)
            consts = cpool.tile([1, 4], FP32)

            def load_consts():
                if cfg["dma_order"] == 0:
                    nc.sync.dma_start(f2c[:], f2c_d[:])
                    nc.sync.dma_start(f1bc2[:], f1r_d[:])
                    nc.sync.dma_start(
                        wtg[:], wtg_d.ap().rearrange("b p c o -> p b c o"))
                    nc.sync.dma_start(consts[:], consts_d[:])
                elif cfg["dma_order"] == 3:
                    # f1bc on the ACT HWDGE queue in parallel with SP's f2c
                    nc.scalar.dma_start(f1bc2[:], f1r_d[:])
                    nc.sync.dma_start(f2c[:], f2c_d[:])
                    nc.sync.dma_start(
                        wtg[:], wtg_d.ap().rearrange("b p c o -> p b c o"))
                    nc.sync.dma_start(consts[:], consts_d[:])
                elif cfg["dma_order"] == 1:
                    nc.sync.dma_start(f1bc2[:], f1r_d[:])
                    nc.sync.dma_start(f2c[:], f2c_d[:])
                    nc.sync.dma_start(
                        wtg[:], wtg_d.ap().rearrange("b p c o -> p b c o"))
                    nc.sync.dma_start(consts[:], consts_d[:])
                else:
                    # order 2: consts emitted AFTER load0 (see loop); here only
                    # the fast ones first
                    nc.sync.dma_start(
                        wtg[:], wtg_d.ap().rearrange("b p c o -> p b c o"))
                    nc.sync.dma_start(f2c[:], f2c_d[:])
                    nc.sync.dma_start(f1bc2[:], f1r_d[:])
                    nc.sync.dma_start(consts[:], consts_d[:])

            constbc = cpool.tile([128, 4], FP32)
            ones_sb = cpool.tile([128, B, 2, 16], SBD)
            nc.gpsimd.memset(ones_sb[:], 1.0)
            one_f = cpool.tile([1, 1], FP32)
            nc.gpsimd.memset(one_f[:], 1.0)
            onesbc = cpool.tile([128, CW], FP32)
            nc.gpsimd.memset(onesbc[:], 1.0)

            with (
                tc.tile_pool(name="psV", bufs=1, space="PSUM") as psV,
                tc.tile_pool(name="psF", bufs=cfg["bufs_fppD"], space="PSUM") as psF,
                tc.tile_pool(name="psS", bufs=1, space="PSUM") as psS,
            ):
                # valsT packed [128o, 6q, 256i]: q = b*3+c, two q per psum bank.
                # start=True (whole-bank clear) only on even q at p==0.
                valsTq = psV.tile([128, 2 * NCH, CW], FP32, name="vT", tag="vT")
                s_acc = psS.tile([1, B, O], FP32, name="sacc", tag="sacc")

                f8_tiles = {}
                tc_tiles = {}

                def stage_load(k):
                    # seqT slice [2b, 2hc, 128h, 512n] -> sT [128h, 2b, 2hc, 512n]
                    sT = p_sT.tile([128, B, 2, 512], BF16, name="sT", tag="sT")
                    if k == 0 and cfg["split_load0"]:
                        # two half-loads so proj(0, pp=0) starts sooner
                        for h in range(2):
                            nc.sync.dma_start(
                                sT[:, :, :, h * 256:(h + 1) * 256],
                                seqT_d[:, :, :, h * 256:(h + 1) * 256]
                                .rearrange("b c h n -> h b c n"),
                            )
                    else:
                        nc.sync.dma_start(
                            sT[:],
                            seqT_d[:, :, :, k * 512:(k + 1) * 512].rearrange(
                                "b c h n -> h b c n"
                            ),
                        )
                    return sT

                def proj_mm(k, sT):
                    # fppD [128n, 2pp, 2b, 2sub, 128o] accumulated over hc;
                    # each pp-slice is exactly one psum bank.
                    fppD = psF.tile([128, 2, B, 2, O], FP32, name="fppD", tag="fppD")
                    for pp in range(2):
                        first = True
                        for b in range(B):
                            for sub in range(2):
                                nsl = slice((2 * pp + sub) * 128,
                                            (2 * pp + sub + 1) * 128)
                                for hc in range(2):
                                    nc.tensor.matmul(
                                        fppD[:, pp, b, sub],
                                        lhsT=sT[:, b, hc, nsl],
                                        rhs=wtg[:, b, hc],
                                        start=first, stop=(hc == 1),
                                        skip_group_check=True,
                                    )
                                    first = False
                    return fppD

                def proj_copy(k, fppD):
                    # psum -> sbuf cast (one op per block), mostly on ACT
                    f8 = p_f8.tile([128, 2, B, 2, O], SBD, name="f8", tag="f8")
                    if k in DVE_COPY_BLOCKS:
                        nc.vector.tensor_copy(f8[:], fppD[:])
                    else:
                        nc.scalar.activation(f8[:], fppD[:], AF.Copy)
                    # colsum accumulation (independent of scores; closes early)
                    for pp in range(2):
                        for b in range(B):
                            if dr8:
                                nc.tensor.matmul(
                                    s_acc[:, b], lhsT=ones_sb[:, b, :, 0:1],
                                    rhs=f8[:, pp, b],
                                    start=(k == 0 and pp == 0 and b == 0),
                                    stop=(k == NBLK - 1 and pp == 1),
                                    perf_mode=DR, skip_group_check=True,
                                )
                            else:
                                for sub in range(2):
                                    nc.tensor.matmul(
                                        s_acc[:, b], lhsT=ones_sb[:, b, sub, 0:1],
                                        rhs=f8[:, pp, b, sub],
                                        start=(k == 0 and pp == 0 and b == 0
                                               and sub == 0),
                                        stop=(k == NBLK - 1 and pp == 1
                                              and sub == 1),
                                        skip_group_check=True,
                                    )
                    f8_tiles[k] = f8

                tc_last = cpool.tile([128, 4, NS], SBD)

                def stage_scores(k):
                    # d [128j, 4jt, 768i] fp32; one custom-dve op per j-tile,
                    # one tanh for the whole block. The LAST block's scores are
                    # hoisted early (into a dedicated tc tile) so the tail only
                    # waits on its projection, not the whole d/tanh chain.
                    jt0 = 4 * k
                    d = p_d.tile([128, 4, NS], FP32, name="d", tag="d")
                    for sub in range(4):
                        nc.vector._custom_dve(
                            diff_lrelu,
                            out=d[:, sub],
                            in0=f1bc2[:, 0:NS],
                            in1=f1bc2[:, NS:2 * NS],
                            s0=f2c[:, 0, jt0 + sub:jt0 + sub + 1],
                            s1=f2c[:, 1, jt0 + sub:jt0 + sub + 1],
                            imm2=0.01,
                        )
                    if k == NBLK - 1:
                        t = tc_last
                    else:
                        t = p_tc.tile([128, 4, NS], SBD, name="tc", tag="tc")
                    nc.scalar.activation(t[:], d[:], AF.Tanh)
                    tc_tiles[k] = (t, 0)

                def stage_b(k):
                    f8 = f8_tiles.pop(k)
                    t, toff = tc_tiles.pop(k)
                    # block NBLK-1 (hoisted scores) is EMITTED before NBLK-2,
                    # so the accumulation stop rides block NBLK-2
                    last = k == (NBLK - 2 if cfg["swap_last_b"] else NBLK - 1)
                    # chunk-major so the last block closes accumulators one
                    # chunk at a time (finalize starts during remaining DRs).
                    # start=True clears a whole psum bank, so only the FIRST
                    # q emitted into each bank (emission order q0,q3,q1,q4,q2,q5
                    # -> banks {0:q0, 1:q3, 2:q4}) may issue it.
                    START_Q = {0, 3, 4}
                    for c in range(NCH):
                        for b in range(B):
                            q = b * NCH + c
                            for pp in range(2):
                                if dr8:
                                    nc.tensor.matmul(
                                        valsTq[:, q],
                                        lhsT=f8[:, pp, b],
                                        rhs=t[:, toff + 2 * pp:toff + 2 * pp + 2,
                                              c * CW:(c + 1) * CW],
                                        start=(k == 0 and pp == 0
                                               and q in START_Q),
                                        stop=(last and pp == 1),
                                        perf_mode=DR, skip_group_check=True,
                                    )
                                else:
                                    for sub in range(2):
                                        nc.tensor.matmul(
                                            valsTq[:, q],
                                            lhsT=f8[:, pp, b, sub],
                                            rhs=t[:, toff + 2 * pp + sub,
                                                  c * CW:(c + 1) * CW],
                                            start=(k == 0 and pp == 0 and sub == 0
                                                   and q in START_Q),
                                            stop=(last and pp == 1 and sub == 1),
                                            skip_group_check=True,
                                        )

                # ---------------- finalize ----------------
                # scol prep happens as soon as s_acc closes (last copy, well
                # before the last DR). y[b] = sgn[b]*P[b] + scol[b];
                # out = relu(y) - 1 + exp(min(y,0)); sgn: +1 (b=0), -1 (b=1).
                scol = cpool.tile([128, B, 3], FP32)   # [:, b, {s, s-1, -s}]

                def fin_prep():
                    # s_acc[0] = Sh0, s_acc[1] = -Sh1 (negated weights);
                    # scol[b] = sgn_acc*s_acc[b] + bias = Sh[b] + bias for b=0
                    # and  -(-Sh1) ... we need scol1 = Sh1 + bias, and the
                    # accumulator already holds -P1, so y1 = valsT1 + scol1.
                    s_sb = p_fin.tile([1, B, O], FP32, tag="fin_ssb")
                    nc.vector.tensor_copy(s_sb[:], s_acc[:])
                    for b in range(B):
                        colp = psF.tile([128, 1], FP32, name="colp", tag="fppD")
                        nc.tensor.matmul(
                            colp[:], lhsT=s_sb[0:1, b], rhs=one_f[:],
                            start=True, stop=True,
                        )
                        sgn = 1.0 if b == 0 else -1.0
                        # scol = sgn*colp + bias ; sm1 = scol - 1
                        nc.vector.tensor_scalar(
                            scol[:, b, 0:1], colp[:], sgn, constbc[:, 0:1],
                            ALU.mult, ALU.add
                        )
                        nc.vector.tensor_scalar(
                            scol[:, b, 1:2], scol[:, b, 0:1], -1.0, None, ALU.add
                        )
                        nc.vector.tensor_scalar(
                            scol[:, b, 2:3], scol[:, b, 0:1], -1.0, None, ALU.mult
                        )

                def fin_chunk(c):
                    # uniform: y = P' + s; out = max(y-1,-1) + exp(min(y,0))
                    # units with index >= 6 - fin_act_units use the ACT path
                    # (ry/nm/e) to take load off DVE's post-loop stream.
                    for b in range(B):
                        q = b * NCH + c
                        src = valsTq[:, q]
                        unit = c * B + b
                        e = p_fin.tile([128, CW], FP32, tag="fin_e")
                        o = p_fin.tile([128, CW], FP32, tag="fin_o")
                        if cfg["fin_nm_act"] and unit % 2 == 1:
                            # balanced split: DVE does rym, ACT does nm+e
                            rym = p_fin.tile([128, CW], FP32, tag="fin_rym")
                            nc.vector.tensor_scalar(
                                rym[:], src, scol[:, b, 1:2], -1.0,
                                ALU.add, ALU.max)
                            nm = p_fin.tile([128, CW], FP32, tag="fin_nm")
                            nc.scalar.activation(
                                nm[:], src, AF.Relu, bias=scol[:, b, 2:3],
                                scale=-1.0)
                            nc.scalar.activation(e[:], nm[:], AF.Exp, scale=-1.0)
                            nc.gpsimd.tensor_tensor(o[:], rym[:], e[:], ALU.add)
                            nc.sync.dma_start(
                                out_d[b, :, c * CW:(c + 1) * CW], o[:])
                            continue
                        if unit >= 2 * NCH - cfg["fin_act_units"]:
                            ry = p_fin.tile([128, CW], FP32, tag="fin_ry")
                            nc.scalar.activation(
                                ry[:], src, AF.Relu, bias=scol[:, b, 0:1])
                            nm = p_fin.tile([128, CW], FP32, tag="fin_nm")
                            nc.scalar.activation(
                                nm[:], src, AF.Relu, bias=scol[:, b, 2:3],
                                scale=-1.0)
                            nc.scalar.activation(e[:], nm[:], AF.Exp, scale=-1.0)
                            t1 = p_fin.tile([128, CW], FP32, tag="fin_t1")
                            nc.gpsimd.tensor_tensor(t1[:], ry[:], e[:], ALU.add)
                            nc.vector.tensor_scalar(o[:], t1[:], -1.0, None,
                                                    ALU.add)
                        else:
                            rym = p_fin.tile([128, CW], FP32, tag="fin_rym")
                            nc.vector.tensor_scalar(
                                rym[:], src, scol[:, b, 1:2], -1.0,
                                ALU.add, ALU.max)
                            m = p_fin.tile([128, CW], FP32, tag="fin_m")
                            nc.vector.tensor_scalar(
                                m[:], src, scol[:, b, 0:1], 0.0,
                                ALU.add, ALU.min)
                            nc.scalar.activation(e[:], m[:], AF.Exp)
                            nc.gpsimd.tensor_tensor(o[:], rym[:], e[:], ALU.add)
                        nc.sync.dma_start(out_d[b, :, c * CW:(c + 1) * CW], o[:])

                def fin_store():
                    pass

                # ---- software pipeline over blocks ----
                # PE p-state warmup: dummy matmuls into s_acc (cleared by the
                # real S-group's start=True) keep PE continuously busy from
                # ~t=0.5us so the first projections run at full clock.
                if cfg["pe_warmup"]:
                    ob = onesbc[:].bitcast(mybir.dt.float32r)
                    for _ in range(cfg["pe_warmup"]):
                        nc.tensor.matmul(
                            s_acc[:, 0], lhsT=ob[:, 0:1], rhs=ob[:, 0:O],
                            start=True, stop=True, skip_group_check=True,
                        )
                lag = max(1, min(cfg["lag"], NBLK))
                sT_tiles = {}
                fppD_tiles = {}
                if cfg["dma_order"] == 2:
                    sT_tiles[0] = stage_load(0)
                    load_consts()
                else:
                    load_consts()
                    sT_tiles[0] = stage_load(0)
                nc.gpsimd.partition_broadcast(constbc[:], consts[:])
                for k in range(NBLK + 1 + lag):
                    if 0 < k < NBLK:
                        sT_tiles[k] = stage_load(k)
                    j = k - 1
                    if 0 <= j < NBLK:
                        fppD_tiles[j] = proj_mm(j, sT_tiles.pop(j))
                        if j in DVE_COPY_BLOCKS:
                            # keep DVE's always-ready d-ops ahead of its copy
                            if j != NBLK - 1:
                                stage_scores(j)
                            proj_copy(j, fppD_tiles.pop(j))
                        else:
                            proj_copy(j, fppD_tiles.pop(j))
                            if j != NBLK - 1:
                                stage_scores(j)
                        if j == 1:
                            stage_scores(NBLK - 1)
                    if k == NBLK:
                        fin_prep()
                    if k >= 1 + lag:
                        j2 = k - 1 - lag
                        if cfg["swap_last_b"] and j2 >= NBLK - 2:
                            j2 = (2 * NBLK - 3) - j2   # 10<->11
                        stage_b(j2)
                if cfg["fin_wide"]:
                    # b's chunks are contiguous in valsTq: one [128,768] view
                    os_ = []
                    for b in range(B):
                        srcv = valsTq[:, 3 * b:3 * b + 3]
                        rym = p_finw.tile([128, NS], FP32, tag="fin_wrym")
                        nc.vector.tensor_scalar(
                            rym[:], srcv, scol[:, b, 1:2], -1.0,
                            ALU.add, ALU.max)
                        m = p_finw.tile([128, NS], FP32, tag="fin_wm")
                        nc.vector.tensor_scalar(
                            m[:], srcv, scol[:, b, 0:1], 0.0,
                            ALU.add, ALU.min)
                        e = p_finw.tile([128, NS], FP32, tag="fin_we")
                        nc.scalar.activation(e[:], m[:], AF.Exp)
                        o = p_finw.tile([128, NS], FP32, tag="fin_wo")
                        if b == 0:
                            nc.gpsimd.tensor_tensor(o[:], rym[:], e[:], ALU.add)
                        else:
                            nc.vector.tensor_tensor(o[:], rym[:], e[:], ALU.add)
                        nc.sync.dma_start(out_d[b], o[:])
                else:
                    for c in range(NCH):
                        fin_chunk(c)
                    fin_store()

    nc.compile()
    return nc


def make_in_maps(seq, W_fts, f1_w, f1_b, f2_w, f2_b, bias):
    seq = np.asarray(seq, dtype=np.float32)
    W = np.asarray(W_fts, dtype=np.float32)
    f1_w = np.asarray(f1_w, dtype=np.float32).reshape(-1)
    f2_w = np.asarray(f2_w, dtype=np.float32).reshape(-1)
    f1_bs = float(np.asarray(f1_b).reshape(-1)[0])
    f2_bs = float(np.asarray(f2_b).reshape(-1)[0])
    bs = float(np.asarray(bias).reshape(-1)[0])

    WT = np.ascontiguousarray(W.T)                      # [H, O]
    g1 = WT @ f1_w                                      # [H]
    g2 = WT @ f2_w

    # seqT [2b, 2hc, 128h, N] bf16
    seqT = np.ascontiguousarray(
        seq.transpose(0, 2, 1).reshape(B, 2, 128, N)
    ).astype(ml_dtypes.bfloat16)
    # wtg [2b, 128h, 2hc, 128o] bf16 = +/-0.5*WT (b=1 negated)
    wtg_half = (0.5 * WT).reshape(2, 128, O).transpose(1, 0, 2)   # [128, 2, O]
    wtg = np.ascontiguousarray(
        np.stack([wtg_half, -wtg_half], axis=0)
    ).astype(ml_dtypes.bfloat16)
    # f2h[b, j] = 0.5 * seq[b] @ g2  (fp32, no bias — folded into f1h)
    f2h = 0.5 * np.einsum("bnh,h->bn", seq, g2)         # [B, N]
    f2c = np.ascontiguousarray(
        f2h.reshape(B, N // 128, 128).transpose(2, 0, 1)
    ).astype(np.float32)                                # [128, B, 48]
    # f1h[b, i] = 0.5 * (seq[b] @ g1 + f1_b + f2_b), own shard per core
    f1h = 0.5 * (np.einsum("bnh,h->bn", seq, g1) + f1_bs + f2_bs)  # [B, N]
    consts = np.array([[bs, bs - 1.0, 0.0, 0.0]], np.float32)

    in_maps = []
    for c in range(NCORES):
        f1dt = ml_dtypes.bfloat16 if _F1BC_BF16[0] else np.float32
        f1r = np.ascontiguousarray(np.broadcast_to(
            f1h[:, c * NS:(c + 1) * NS].reshape(1, B * NS), (128, B * NS)
        )).astype(f1dt)
        in_maps.append({
            "seqT": seqT,
            "wtg": wtg,
            "f2c": f2c,
            "f1r": f1r,
            "consts": consts,
        })
    return in_maps


_NC_CACHE = []
_F1BC_BF16 = [DEFAULT_CFG["f1bc_bf16"]]


def kernel(seq, W_fts, f1_w, f1_b, f2_w, f2_b, bias):
    if not _NC_CACHE:
        _NC_CACHE.append(build_nc())
    nc = _NC_CACHE[0]
    in_maps = make_in_maps(seq, W_fts, f1_w, f1_b, f2_w, f2_b, bias)
    res = run_bass_kernel_spmd(nc, in_maps, core_ids=list(range(NCORES)))
    # per-core out is [B, O, NS] (transposed); gather + host-transpose
    outT = np.concatenate(
        [res.results[c]["out"] for c in range(NCORES)], axis=2
    )                                                    # [B, O, N]
    return np.ascontiguousarray(outT.transpose(0, 2, 1))


# revision 7
# speedup vs baseline: 1.0140x; 1.0140x over previous
"""Trainium2 Bass kernel v2 for the GAT-style attention head (B=2, N=6144, H=256, O=128).

Math (matching the reference):
  seq_fts = seq @ W_fts.T                       [B, N, O]
  f1 = seq_fts @ f1_w + f1_b                    [B, N]
  f2 = seq_fts @ f2_w + f2_b                    [B, N]
  z[b, j, i]  = leaky_relu(f1[b, i] + f2[b, j], 0.01)
  coefs[b,j,i] = softmax_b(z)   (B=2: c0 = sigmoid(z0 - z1), c1 = 1 - c0)
  vals[b, i, o] = sum_j coefs[b, j, i] * seq_fts[b, j, o]
  out = elu(vals + bias)

Key reformulation: c0 - 0.5 = 0.5*tanh((z0-z1)/2), and leaky_relu is
positively homogeneous, so with HALVED f1/f2 (0.5 folded into the host-packed
weights) the device computes
  dh[j,i] = lrelu(f1h[i]+f2h[j])|b0 - lrelu(...)|b1     (fused custom DVE op)
  tc = tanh(dh)                                          (ACT, fp8 out)
  P[b]  = sum_j tc[j,i] * fts_half[b,j,o]                (fp8 DoubleRow matmuls,
                                                          transposed acc [o, i])
  vals[0] = Sh[0] + P[0],  vals[1] = Sh[1] - P[1]        (Sh = colsum fts_half)
  out = elu(vals + bias) = relu(y) - 1 + exp(min(y, 0))

Sharding: each core owns 768 output rows i; seq streams fully through every
core (j loop). Host prep: seq pre-transposed/cast to bf16 [2b,2hc,128h,N],
f1h (own shard) / f2h (all j) computed on host fp32, W^T*0.5 packed bf16.
Output written transposed [B, O, NS] and fixed up on the host.

Schedule: 12 blocks of 2 j-tile-pairs (512 j rows each). Steady state is
DVE-bound (4 fused lrelu-diff ops per block); psum->sbuf fts casts ride
mostly on ACT (cfg dve_copy_every), tanh is one ACT op per block.
"""

import numpy as np
import ml_dtypes

import concourse.bacc as bacc
import concourse.bass as bass
import concourse.mybir as mybir
import concourse.tile as tile
from concourse.bass_utils import run_bass_kernel_spmd

B, N, H, O = 2, 6144, 256, 128
NCORES = 8
NS = N // NCORES          # 768 i-rows per core
NBLK = N // 512           # 12 blocks, each 2 pairs = 4 j-tiles = 512 j rows
FP32 = mybir.dt.float32
BF16 = mybir.dt.bfloat16
FP8 = mybir.dt.float8e4
AF = mybir.ActivationFunctionType
ALU = mybir.AluOpType
DR = mybir.MatmulPerfMode.DoubleRow

_DVE_OP_NAME = "DIFF_LRELU_ANT"

DEFAULT_CFG = dict(
    lag=1,                # stage-B lags scores by this many blocks
    bufs_sT=3,
    bufs_fppD=2,
    bufs_f8=3,
    bufs_d=3,
    bufs_tc=3,
    stageb="dr8",         # dr8 | bf16
    dve_copy_blocks=(9, 10, 11),  # blocks whose fts cast runs on DVE (rest ACT)
    split_tanh_block=-1,  # block whose tanh is emitted per-subtile
    store_queues=1,
    fin_act_units=0,
    pe_warmup=16,
    fin_wide=3,
    split_load0=True,
    swap_last_b=True,
    fin_nm_act=False,
    f1bc_bf16=True,
    dma_order=0,
    fin_dve_units=3,      # finalize units (of 6) using the DVE ry/m path
)


def _get_diff_lrelu_op():
    """Register (once) and return the fused custom DVE op:
    out = lrelu(in0 + s0) - lrelu(in1 + s1), slope imm2."""
    import concourse.dve_ops as dve_ops
    from concourse.dve_ops import OPS, DveOp

    for op in OPS:
        if op.name == _DVE_OP_NAME:
            return op

    from concourse.dve_spec import C0, C1, C2, Spec, Src0, Src1, lower, maxx
    from concourse.dve_uop import DveOpSpec

    a = Src0 + C0
    b = Src1 + C1
    spec = Spec(
        body=maxx(a, a * C2) - maxx(b, b * C2),
        reference=lambda in0, in1, s0, s1, imm2: (
            np.maximum(in0 + s0, (in0 + s0) * imm2)
            - np.maximum(in1 + s1, (in1 + s1) * imm2)
        ).astype(np.float32),
    )
    row = dve_ops._CUSTOM_DVE_ROW_BASE + len(OPS)
    shas = {}
    for ver in ("v3",):
        uops = lower(spec, ver=ver)
        shas[ver] = DveOpSpec(
            name=_DVE_OP_NAME, opcode=row, uops=uops, rd1_en=True
        ).sha(ver)
    op = DveOp(_DVE_OP_NAME, spec, subdim=False, uops_sha=shas)
    OPS.append(op)
    dve_ops.CUSTOM_DVE_SPECS[_DVE_OP_NAME] = spec
    dve_ops._SUB_OPCODE_FOR_NAME[_DVE_OP_NAME] = row
    return op


def build_nc(cfg=None):
    cfg = {**DEFAULT_CFG, **(cfg or {})}
    diff_lrelu = _get_diff_lrelu_op()
    dr8 = cfg["stageb"] == "dr8"
    SBD = FP8 if dr8 else BF16     # stage-B operand dtype
    DVE_COPY_BLOCKS = set(cfg["dve_copy_blocks"])

    nc = bacc.Bacc("TRN2", target_bir_lowering=False, debug=False, num_devices=NCORES)

    # seqT: [2b, 2hc, 128h, N] bf16 (host-transposed)
    seqT_d = nc.declare_dram_parameter("seqT", [B, 2, 128, N], BF16, isOutput=False)
    # wtg: [2b, 128h, 2hc, 128o] bf16 = +/-0.5 * W^T chunks (b=1 negated so
    # the accumulator holds -P1 and finalize is uniform y = P' + s)
    wtg_d = nc.declare_dram_parameter("wtg", [B, 128, 2, O], BF16, isOutput=False)
    # f2h columns for all j: [128j, 2b, 48jt] fp32
    f2c_d = nc.declare_dram_parameter("f2c", [128, B, N // 128], FP32, isOutput=False)
    # f1h broadcast for own shard (bias folded in): [128, 2b*768]
    F1DT = BF16 if cfg["f1bc_bf16"] else FP32
    f1r_d = nc.declare_dram_parameter("f1r", [128, B * NS], F1DT, isOutput=False)
    # consts: [bias, bias-1, 0, 0]
    consts_d = nc.declare_dram_parameter("consts", [1, 4], FP32, isOutput=False)
    # transposed output [B, O, NS]
    out_d = nc.declare_dram_parameter("out", [B, O, NS], FP32, isOutput=True)

    NCH = 3                      # accumulator column chunks (2 per psum bank)
    CW = NS // NCH               # 256

    with tile.TileContext(nc) as tc:
        with (
            tc.tile_pool(name="const", bufs=1) as cpool,
            tc.tile_pool(name="sT", bufs=cfg["bufs_sT"]) as p_sT,
            tc.tile_pool(name="fts8", bufs=cfg["bufs_f8"]) as p_f8,
            tc.tile_pool(name="dt", bufs=cfg["bufs_d"]) as p_d,
            tc.tile_pool(name="tct", bufs=cfg["bufs_tc"]) as p_tc,
            tc.tile_pool(name="fin", bufs=14) as p_fin,
            tc.tile_pool(name="finw", bufs=1) as p_finw,
        ):
            # ---------------- constants / setup ----------------
            wtg = cpool.tile([128, B, 2, O], BF16)
            f1bc2 = cpool.tile([128, B * NS], F1DT)
            f2c = cpool.tile([128, B, N // 128], FP32)
            consts = cpool.tile([1, 4], FP32)

            def load_consts():
                if cfg["dma_order"] == 0:
                    nc.sync.dma_start(f2c[:], f2c_d[:])
                    nc.sync.dma_start(f1bc2[:], f1r_d[:])
                    nc.sync.dma_start(
                        wtg[:], wtg_d.ap().rearrange("b p c o -> p b c o"))
                    nc.sync.dma_start(consts[:], consts_d[:])
                elif cfg["dma_order"] == 3:
                    # f1bc on the ACT HWDGE queue in parallel with SP's f2c
                    nc.scalar.dma_start(f1bc2[:], f1r_d[:])
                    nc.sync.dma_start(f2c[:], f2c_d[:])
                    nc.sync.dma_start(
                        wtg[:], wtg_d.ap().rearrange("b p c o -> p b c o"))
                    nc.sync.dma_start(consts[:], consts_d[:])
                elif cfg["dma_order"] == 1:
                    nc.sync.dma_start(f1bc2[:], f1r_d[:])
                    nc.sync.dma_start(f2c[:], f2c_d[:])
                    nc.sync.dma_start(
                        wtg[:], wtg_d.ap().rearrange("b p c o -> p b c o"))
                    nc.sync.dma_start(consts[:], consts_d[:])
                else:
                    # order 2: consts emitted AFTER load0 (see loop); here only
                    # the fast ones first
                    nc.sync.dma_start(
                        wtg[:], wtg_d.ap().rearrange("b p c o -> p b c o"))
                    nc.sync.dma_start(f2c[:], f2c_d[:])
                    nc.sync.dma_start(f1bc2[:], f1r_d[:])
                    nc.sync.dma_start(consts[:], consts_d[:])

            constbc = cpool.tile([128, 4], FP32)
            ones_sb = cpool.tile([128, B, 2, 16], SBD)
            nc.gpsimd.memset(ones_sb[:], 1.0)
            one_f = cpool.tile([1, 1], FP32)
            nc.gpsimd.memset(one_f[:], 1.0)
            onesbc = cpool.tile([128, CW], FP32)
            nc.gpsimd.memset(onesbc[:], 1.0)

            with (
                tc.tile_pool(name="psV", bufs=1, space="PSUM") as psV,
                tc.tile_pool(name="psF", bufs=cfg["bufs_fppD"], space="PSUM") as psF,
                tc.tile_pool(name="psS", bufs=1, space="PSUM") as psS,
            ):
                # valsT packed [128o, 6q, 256i]: q = b*3+c, two q per psum bank.
                # start=True (whole-bank clear) only on even q at p==0.
                valsTq = psV.tile([128, 2 * NCH, CW], FP32, name="vT", tag="vT")
                s_acc = psS.tile([1, B, O], FP32, name="sacc", tag="sacc")

                f8_tiles = {}
                tc_tiles = {}

                def stage_load(k):
                    # seqT slice [2b, 2hc, 128h, 512n] -> sT [128h, 2b, 2hc, 512n]
                    sT = p_sT.tile([128, B, 2, 512], BF16, name="sT", tag="sT")
                    if k == 0 and cfg["split_load0"]:
                        # two half-loads so proj(0, pp=0) starts sooner
                        for h in range(2):
                            nc.sync.dma_start(
                                sT[:, :, :, h * 256:(h + 1) * 256],
                                seqT_d[:, :, :, h * 256:(h + 1) * 256]
                                .rearrange("b c h n -> h b c n"),
                            )
                    else:
                        nc.sync.dma_start(
                            sT[:],
                            seqT_d[:, :, :, k * 512:(k + 1) * 512].rearrange(
                                "b c h n -> h b c n"
                            ),
                        )
                    return sT

                def proj_mm(k, sT):
                    # fppD [128n, 2pp, 2b, 2sub, 128o] accumulated over hc;
                    # each pp-slice is exactly one psum bank.
                    fppD = psF.tile([128, 2, B, 2, O], FP32, name="fppD", tag="fppD")
                    for pp in range(2):
                        first = True
                        for b in range(B):
                            for sub in range(2):
                                nsl = slice((2 * pp + sub) * 128,
                                            (2 * pp + sub + 1) * 128)
                                for hc in range(2):
                                    nc.tensor.matmul(
                                        fppD[:, pp, b, sub],
                                        lhsT=sT[:, b, hc, nsl],
                                        rhs=wtg[:, b, hc],
                                        start=first, stop=(hc == 1),
                                        skip_group_check=True,
                                    )
                                    first = False
                    return fppD

                def proj_copy(k, fppD):
                    # psum -> sbuf cast (one op per block), mostly on ACT
                    f8 = p_f8.tile([128, 2, B, 2, O], SBD, name="f8", tag="f8")
                    if k in DVE_COPY_BLOCKS:
                        nc.vector.tensor_copy(f8[:], fppD[:])
                    else:
                        nc.scalar.activation(f8[:], fppD[:], AF.Copy)
                    # colsum accumulation (independent of scores; closes early)
                    for pp in range(2):
                        for b in range(B):
                            if dr8:
                                nc.tensor.matmul(
                                    s_acc[:, b], lhsT=ones_sb[:, b, :, 0:1],
                                    rhs=f8[:, pp, b],
                                    start=(k == 0 and pp == 0 and b == 0),
                                    stop=(k == NBLK - 1 and pp == 1),
                                    perf_mode=DR, skip_group_check=True,
                                )
                            else:
                                for sub in range(2):
                                    nc.tensor.matmul(
                                        s_acc[:, b], lhsT=ones_sb[:, b, sub, 0:1],
                                        rhs=f8[:, pp, b, sub],
                                        start=(k == 0 and pp == 0 and b == 0
                                               and sub == 0),
                                        stop=(k == NBLK - 1 and pp == 1
                                              and sub == 1),
                                        skip_group_check=True,
                                    )
                    f8_tiles[k] = f8

                tc_last = cpool.tile([128, 4, NS], SBD)

                def stage_scores(k):
                    # d [128j, 4jt, 768i] fp32; one custom-dve op per j-tile,
                    # one tanh for the whole block. The LAST block's scores are
                    # hoisted early (into a dedicated tc tile) so the tail only
                    # waits on its projection, not the whole d/tanh chain.
                    jt0 = 4 * k
                    d = p_d.tile([128, 4, NS], FP32, name="d", tag="d")
                    for sub in range(4):
                        nc.vector._custom_dve(
                            diff_lrelu,
                            out=d[:, sub],
                            in0=f1bc2[:, 0:NS],
                            in1=f1bc2[:, NS:2 * NS],
                            s0=f2c[:, 0, jt0 + sub:jt0 + sub + 1],
                            s1=f2c[:, 1, jt0 + sub:jt0 + sub + 1],
                            imm2=0.01,
                        )
                    if k == NBLK - 1:
                        t = tc_last
                    else:
                        t = p_tc.tile([128, 4, NS], SBD, name="tc", tag="tc")
                    nc.scalar.activation(t[:], d[:], AF.Tanh)
                    tc_tiles[k] = (t, 0)

                def stage_b(k):
                    f8 = f8_tiles.pop(k)
                    t, toff = tc_tiles.pop(k)
                    # block NBLK-1 (hoisted scores) is EMITTED before NBLK-2,
                    # so the accumulation stop rides block NBLK-2
                    last = k == (NBLK - 2 if cfg["swap_last_b"] else NBLK - 1)
                    # chunk-major so the last block closes accumulators one
                    # chunk at a time (finalize starts during remaining DRs).
                    # start=True clears a whole psum bank, so only the FIRST
                    # q emitted into each bank (emission order q0,q3,q1,q4,q2,q5
                    # -> banks {0:q0, 1:q3, 2:q4}) may issue it.
                    START_Q = {0, 3, 4}
                    for c in range(NCH):
                        for b in range(B):
                            q = b * NCH + c
                            for pp in range(2):
                                if dr8:
                                    nc.tensor.matmul(
                                        valsTq[:, q],
                                        lhsT=f8[:, pp, b],
                                        rhs=t[:, toff + 2 * pp:toff + 2 * pp + 2,
                                              c * CW:(c + 1) * CW],
                                        start=(k == 0 and pp == 0
                                               and q in START_Q),
                                        stop=(last and pp == 1),
                                        perf_mode=DR, skip_group_check=True,
                                    )
                                else:
                                    for sub in range(2):
                                        nc.tensor.matmul(
                                            valsTq[:, q],
                                            lhsT=f8[:, pp, b, sub],
                                            rhs=t[:, toff + 2 * pp + sub,
                                                  c * CW:(c + 1) * CW],
                                            start=(k == 0 and pp == 0 and sub == 0
                                                   and q in START_Q),
                                            stop=(last and pp == 1 and sub == 1),
                                            skip_group_check=True,
                                        )

                # ---------------- finalize ----------------
                # scol prep happens as soon as s_acc closes (last copy, well
                # before the last DR). y[b] = sgn[b]*P[b] + scol[b];
                # out = relu(y) - 1 + exp(min(y,0)); sgn: +1 (b=0), -1 (b=1).
                scol = cpool.tile([128, B, 3], FP32)   # [:, b, {s, s-1, -s}]

                def fin_prep():
                    # s_acc[0] = Sh0, s_acc[1] = -Sh1 (negated weights);
                    # scol[b] = sgn_acc*s_acc[b] + bias = Sh[b] + bias for b=0
                    # and  -(-Sh1) ... we need scol1 = Sh1 + bias, and the
                    # accumulator already holds -P1, so y1 = valsT1 + scol1.
                    s_sb = p_fin.tile([1, B, O], FP32, tag="fin_ssb")
                    nc.vector.tensor_copy(s_sb[:], s_acc[:])
                    for b in range(B):
                        colp = psF.tile([128, 1], FP32, name="colp", tag="fppD")
                        nc.tensor.matmul(
                            colp[:], lhsT=s_sb[0:1, b], rhs=one_f[:],
                            start=True, stop=True,
                        )
                        sgn = 1.0 if b == 0 else -1.0
                        # scol = sgn*colp + bias ; sm1 = scol - 1
                        nc.vector.tensor_scalar(
                            scol[:, b, 0:1], colp[:], sgn, constbc[:, 0:1],
                            ALU.mult, ALU.add
                        )
                        nc.vector.tensor_scalar(
                            scol[:, b, 1:2], scol[:, b, 0:1], -1.0, None, ALU.add
                        )
                        nc.vector.tensor_scalar(
                            scol[:, b, 2:3], scol[:, b, 0:1], -1.0, None, ALU.mult
                        )

                def fin_chunk(c):
                    # uniform: y = P' + s; out = max(y-1,-1) + exp(min(y,0))
                    # units with index >= 6 - fin_act_units use the ACT path
                    # (ry/nm/e) to take load off DVE's post-loop stream.
                    for b in range(B):
                        q = b * NCH + c
                        src = valsTq[:, q]
                        unit = c * B + b
                        e = p_fin.tile([128, CW], FP32, tag="fin_e")
                        o = p_fin.tile([128, CW], FP32, tag="fin_o")
                        if cfg["fin_nm_act"] and unit % 2 == 1:
                            # balanced split: DVE does rym, ACT does nm+e
                            rym = p_fin.tile([128, CW], FP32, tag="fin_rym")
                            nc.vector.tensor_scalar(
                                rym[:], src, scol[:, b, 1:2], -1.0,
                                ALU.add, ALU.max)
                            nm = p_fin.tile([128, CW], FP32, tag="fin_nm")
                            nc.scalar.activation(
                                nm[:], src, AF.Relu, bias=scol[:, b, 2:3],
                                scale=-1.0)
                            nc.scalar.activation(e[:], nm[:], AF.Exp, scale=-1.0)
                            nc.gpsimd.tensor_tensor(o[:], rym[:], e[:], ALU.add)
                            nc.sync.dma_start(
                                out_d[b, :, c * CW:(c + 1) * CW], o[:])
                            continue
                        if unit >= 2 * NCH - cfg["fin_act_units"]:
                            ry = p_fin.tile([128, CW], FP32, tag="fin_ry")
                            nc.scalar.activation(
                                ry[:], src, AF.Relu, bias=scol[:, b, 0:1])
                            nm = p_fin.tile([128, CW], FP32, tag="fin_nm")
                            nc.scalar.activation(
                                nm[:], src, AF.Relu, bias=scol[:, b, 2:3],
                                scale=-1.0)
                            nc.scalar.activation(e[:], nm[:], AF.Exp, scale=-1.0)
                            t1 = p_fin.tile([128, CW], FP32, tag="fin_t1")
                            nc.gpsimd.tensor_tensor(t1[:], ry[:], e[:], ALU.add)
                            nc.vector.tensor_scalar(o[:], t1[:], -1.0, None,
                                                    ALU.add)
                        else:
                            rym = p_fin.tile([128, CW], FP32, tag="fin_rym")
                            nc.vector.tensor_scalar(
                                rym[:], src, scol[:, b, 1:2], -1.0,
                                ALU.add, ALU.max)
                            m = p_fin.tile([128, CW], FP32, tag="fin_m")
                            nc.vector.tensor_scalar(
                                m[:], src, scol[:, b, 0:1], 0.0,
                                ALU.add, ALU.min)
                            nc.scalar.activation(e[:], m[:], AF.Exp)
                            nc.gpsimd.tensor_tensor(o[:], rym[:], e[:], ALU.add)
                        nc.sync.dma_start(out_d[b, :, c * CW:(c + 1) * CW], o[:])

                def fin_store():
                    pass

                # ---- software pipeline over blocks ----
                # PE p-state warmup: dummy matmuls into s_acc (cleared by the
                # real S-group's start=True) keep PE continuously busy from
                # ~t=0.5us so the first projections run at full clock.
                if cfg["pe_warmup"]:
                    ob = onesbc[:].bitcast(mybir.dt.float32r)
                    for _ in range(cfg["pe_warmup"]):
                        nc.tensor.matmul(
                            s_acc[:, 0], lhsT=ob[:, 0:1], rhs=ob[:, 0:O],
                            start=True, stop=True, skip_group_check=True,
                        )
                lag = max(1, min(cfg["lag"], NBLK))
                sT_tiles = {}
                fppD_tiles = {}
                if cfg["dma_order"] == 2:
                    sT_tiles[0] = stage_load(0)
                    load_consts()
                else:
                    load_consts()
                    sT_tiles[0] = stage_load(0)
                nc.gpsimd.partition_broadcast(constbc[:], consts[:])
                for k in range(NBLK + 1 + lag):
                    if 0 < k < NBLK:
                        sT_tiles[k] = stage_load(k)
                    j = k - 1
                    if 0 <= j < NBLK:
                        fppD_tiles[j] = proj_mm(j, sT_tiles.pop(j))
                        if j in DVE_COPY_BLOCKS:
                            # keep DVE's always-ready d-ops ahead of its copy
                            if j != NBLK - 1:
                                stage_scores(j)
                            proj_copy(j, fppD_tiles.pop(j))
                        else:
                            proj_copy(j, fppD_tiles.pop(j))
                            if j != NBLK - 1:
                                stage_scores(j)
                        if j == 1:
                            stage_scores(NBLK - 1)
                    if k == NBLK:
                        fin_prep()
                    if k >= 1 + lag:
                        j2 = k - 1 - lag
                        if cfg["swap_last_b"] and j2 >= NBLK - 2:
                            j2 = (2 * NBLK - 3) - j2   # 10<->11
                        stage_b(j2)
                if cfg["fin_wide"] == 3:
                    # 4 units of [128, 384]: fewer DVE per-op overheads; psum
                    # reads may span q-chunk bank boundaries (non-matmul ok)
                    HW = NS // 2
                    for b in range(B):
                        flat = valsTq[:, 3 * b:3 * b + 3].rearrange(
                            "p a b -> p (a b)")
                        for h in range(2):
                            srcv = flat[:, h * HW:(h + 1) * HW]
                            rym = p_fin.tile([128, HW], FP32, tag="fin_hrym")
                            nc.vector.tensor_scalar(
                                rym[:], srcv, scol[:, b, 1:2], -1.0,
                                ALU.add, ALU.max)
                            m = p_fin.tile([128, HW], FP32, tag="fin_hm")
                            nc.vector.tensor_scalar(
                                m[:], srcv, scol[:, b, 0:1], 0.0,
                                ALU.add, ALU.min)
                            e = p_fin.tile([128, HW], FP32, tag="fin_he")
                            nc.scalar.activation(e[:], m[:], AF.Exp)
                            o = p_fin.tile([128, HW], FP32, tag="fin_ho")
                            if b == 1 and h == 1:
                                nc.vector.tensor_tensor(o[:], rym[:], e[:],
                                                        ALU.add)
                            else:
                                nc.gpsimd.tensor_tensor(o[:], rym[:], e[:],
                                                        ALU.add)
                            nc.sync.dma_start(
                                out_d[b, :, h * HW:(h + 1) * HW], o[:])
                elif cfg["fin_wide"]:
                    # b's chunks are contiguous in valsTq: one [128,768] view
                    os_ = []
                    for b in range(B):
                        srcv = valsTq[:, 3 * b:3 * b + 3]
                        rym = p_finw.tile([128, NS], FP32, tag="fin_wrym")
                        nc.vector.tensor_scalar(
                            rym[:], srcv, scol[:, b, 1:2], -1.0,
                            ALU.add, ALU.max)
                        m = p_finw.tile([128, NS], FP32, tag="fin_wm")
                        nc.vector.tensor_scalar(
                            m[:], srcv, scol[:, b, 0:1], 0.0,
                            ALU.add, ALU.min)
                        e = p_finw.tile([128, NS], FP32, tag="fin_we")
                        nc.scalar.activation(e[:], m[:], AF.Exp)
                        o = p_finw.tile([128, NS], FP32, tag="fin_wo")
                        if b == 0:
                            nc.gpsimd.tensor_tensor(o[:], rym[:], e[:], ALU.add)
                        else:
                            nc.vector.tensor_tensor(o[:], rym[:], e[:], ALU.add)
                        nc.sync.dma_start(out_d[b], o[:])
                else:
                    for c in range(NCH):
                        fin_chunk(c)
                    fin_store()

    nc.compile()
    return nc


def make_in_maps(seq, W_fts, f1_w, f1_b, f2_w, f2_b, bias):
    seq = np.asarray(seq, dtype=np.float32)
    W = np.asarray(W_fts, dtype=np.float32)
    f1_w = np.asarray(f1_w, dtype=np.float32).reshape(-1)
    f2_w = np.asarray(f2_w, dtype=np.float32).reshape(-1)
    f1_bs = float(np.asarray(f1_b).reshape(-1)[0])
    f2_bs = float(np.asarray(f2_b).reshape(-1)[0])
    bs = float(np.asarray(bias).reshape(-1)[0])

    WT = np.ascontiguousarray(W.T)                      # [H, O]
    g1 = WT @ f1_w                                      # [H]
    g2 = WT @ f2_w

    # seqT [2b, 2hc, 128h, N] bf16
    seqT = np.ascontiguousarray(
        seq.transpose(0, 2, 1).reshape(B, 2, 128, N)
    ).astype(ml_dtypes.bfloat16)
    # wtg [2b, 128h, 2hc, 128o] bf16 = +/-0.5*WT (b=1 negated)
    wtg_half = (0.5 * WT).reshape(2, 128, O).transpose(1, 0, 2)   # [128, 2, O]
    wtg = np.ascontiguousarray(
        np.stack([wtg_half, -wtg_half], axis=0)
    ).astype(ml_dtypes.bfloat16)
    # f2h[b, j] = 0.5 * seq[b] @ g2  (fp32, no bias — folded into f1h)
    f2h = 0.5 * np.einsum("bnh,h->bn", seq, g2)         # [B, N]
    f2c = np.ascontiguousarray(
        f2h.reshape(B, N // 128, 128).transpose(2, 0, 1)
    ).astype(np.float32)                                # [128, B, 48]
    # f1h[b, i] = 0.5 * (seq[b] @ g1 + f1_b + f2_b), own shard per core
    f1h = 0.5 * (np.einsum("bnh,h->bn", seq, g1) + f1_bs + f2_bs)  # [B, N]
    consts = np.array([[bs, bs - 1.0, 0.0, 0.0]], np.float32)

    in_maps = []
    for c in range(NCORES):
        f1dt = ml_dtypes.bfloat16 if _F1BC_BF16[0] else np.float32
        f1r = np.ascontiguousarray(np.broadcast_to(
            f1h[:, c * NS:(c + 1) * NS].reshape(1, B * NS), (128, B * NS)
        )).astype(f1dt)
        in_maps.append({
            "seqT": seqT,
            "wtg": wtg,
            "f2c": f2c,
            "f1r": f1r,
            "consts": consts,
        })
    return in_maps


_NC_CACHE = []
_F1BC_BF16 = [DEFAULT_CFG["f1bc_bf16"]]


def kernel(seq, W_fts, f1_w, f1_b, f2_w, f2_b, bias):
    if not _NC_CACHE:
        _NC_CACHE.append(build_nc())
    nc = _NC_CACHE[0]
    in_maps = make_in_maps(seq, W_fts, f1_w, f1_b, f2_w, f2_b, bias)
    res = run_bass_kernel_spmd(nc, in_maps, core_ids=list(range(NCORES)))
    # per-core out is [B, O, NS] (transposed); gather + host-transpose
    outT = np.concatenate(
        [res.results[c]["out"] for c in range(NCORES)], axis=2
    )                                                    # [B, O, N]
    return np.ascontiguousarray(outT.transpose(0, 2, 1))


# revision 8
# speedup vs baseline: 1.0244x; 1.0103x over previous
"""Trainium2 Bass kernel v2 for the GAT-style attention head (B=2, N=6144, H=256, O=128).

Math (matching the reference):
  seq_fts = seq @ W_fts.T                       [B, N, O]
  f1 = seq_fts @ f1_w + f1_b                    [B, N]
  f2 = seq_fts @ f2_w + f2_b                    [B, N]
  z[b, j, i]  = leaky_relu(f1[b, i] + f2[b, j], 0.01)
  coefs[b,j,i] = softmax_b(z)   (B=2: c0 = sigmoid(z0 - z1), c1 = 1 - c0)
  vals[b, i, o] = sum_j coefs[b, j, i] * seq_fts[b, j, o]
  out = elu(vals + bias)

Key reformulation: c0 - 0.5 = 0.5*tanh((z0-z1)/2), and leaky_relu is
positively homogeneous, so with HALVED f1/f2 (0.5 folded into the host-packed
weights) the device computes
  dh[j,i] = lrelu(f1h[i]+f2h[j])|b0 - lrelu(...)|b1     (fused custom DVE op)
  tc = tanh(dh)                                          (ACT, fp8 out)
  P[b]  = sum_j tc[j,i] * fts_half[b,j,o]                (fp8 DoubleRow matmuls,
                                                          transposed acc [o, i])
  vals[0] = Sh[0] + P[0],  vals[1] = Sh[1] - P[1]        (Sh = colsum fts_half)
  out = elu(vals + bias) = relu(y) - 1 + exp(min(y, 0))

Sharding: each core owns 768 output rows i; seq streams fully through every
core (j loop). Host prep: seq pre-transposed/cast to bf16 [2b,2hc,128h,N],
f1h (own shard) / f2h (all j) computed on host fp32, W^T*0.5 packed bf16.
Output written transposed [B, O, NS] and fixed up on the host.

Schedule: 12 blocks of 2 j-tile-pairs (512 j rows each). Steady state is
DVE-bound (4 fused lrelu-diff ops per block); psum->sbuf fts casts ride
mostly on ACT (cfg dve_copy_every), tanh is one ACT op per block.
"""

import numpy as np
import ml_dtypes

import concourse.bacc as bacc
import concourse.bass as bass
import concourse.mybir as mybir
import concourse.tile as tile
from concourse.bass_utils import run_bass_kernel_spmd

B, N, H, O = 2, 6144, 256, 128
NCORES = 8
NS = N // NCORES          # 768 i-rows per core
NBLK = N // 512           # 12 blocks, each 2 pairs = 4 j-tiles = 512 j rows
FP32 = mybir.dt.float32
BF16 = mybir.dt.bfloat16
FP8 = mybir.dt.float8e4
AF = mybir.ActivationFunctionType
ALU = mybir.AluOpType
DR = mybir.MatmulPerfMode.DoubleRow

_DVE_OP_NAME = "DIFF_LRELU_ANT"

DEFAULT_CFG = dict(
    lag=1,                # stage-B lags scores by this many blocks
    bufs_sT=3,
    bufs_fppD=2,
    bufs_f8=3,
    bufs_d=3,
    bufs_tc=3,
    stageb="dr8",         # dr8 | bf16
    dve_copy_blocks=(9, 10, 11),  # blocks whose fts cast runs on DVE (rest ACT)
    split_tanh_block=-1,  # block whose tanh is emitted per-subtile
    store_queues=1,
    fin_act_units=0,
    pe_warmup=16,
    fin_wide=3,
    split_load0=True,
    swap_last_b=True,
    fin_nm_act=False,
    f1bc_bf16=True,
    dma_order=0,
    fin_dve_units=3,      # finalize units (of 6) using the DVE ry/m path
)


def _get_diff_lrelu_op():
    """Register (once) and return the fused custom DVE op:
    out = lrelu(in0 + s0) - lrelu(in1 + s1), slope imm2."""
    import concourse.dve_ops as dve_ops
    from concourse.dve_ops import OPS, DveOp

    for op in OPS:
        if op.name == _DVE_OP_NAME:
            return op

    from concourse.dve_spec import C0, C1, C2, Spec, Src0, Src1, lower, maxx
    from concourse.dve_uop import DveOpSpec

    a = Src0 + C0
    b = Src1 + C1
    spec = Spec(
        body=maxx(a, a * C2) - maxx(b, b * C2),
        reference=lambda in0, in1, s0, s1, imm2: (
            np.maximum(in0 + s0, (in0 + s0) * imm2)
            - np.maximum(in1 + s1, (in1 + s1) * imm2)
        ).astype(np.float32),
    )
    row = dve_ops._CUSTOM_DVE_ROW_BASE + len(OPS)
    shas = {}
    for ver in ("v3",):
        uops = lower(spec, ver=ver)
        shas[ver] = DveOpSpec(
            name=_DVE_OP_NAME, opcode=row, uops=uops, rd1_en=True
        ).sha(ver)
    op = DveOp(_DVE_OP_NAME, spec, subdim=False, uops_sha=shas)
    OPS.append(op)
    dve_ops.CUSTOM_DVE_SPECS[_DVE_OP_NAME] = spec
    dve_ops._SUB_OPCODE_FOR_NAME[_DVE_OP_NAME] = row
    return op


def build_nc(cfg=None):
    cfg = {**DEFAULT_CFG, **(cfg or {})}
    diff_lrelu = _get_diff_lrelu_op()
    dr8 = cfg["stageb"] == "dr8"
    SBD = FP8 if dr8 else BF16     # stage-B operand dtype
    DVE_COPY_BLOCKS = set(cfg["dve_copy_blocks"])

    nc = bacc.Bacc("TRN2", target_bir_lowering=False, debug=False, num_devices=NCORES)

    # seqT: [2b, 2hc, 128h, N] bf16 (host-transposed)
    seqT_d = nc.declare_dram_parameter("seqT", [B, 2, 128, N], BF16, isOutput=False)
    # wtg: [2b, 128h, 2hc, 128o] bf16 = +/-0.5 * W^T chunks (b=1 negated so
    # the accumulator holds -P1 and finalize is uniform y = P' + s)
    wtg_d = nc.declare_dram_parameter("wtg", [B, 128, 2, O], BF16, isOutput=False)
    # f2h columns for all j: [128j, 2b, 48jt] fp32
    f2c_d = nc.declare_dram_parameter("f2c", [128, B, N // 128], FP32, isOutput=False)
    # f1h broadcast for own shard (bias folded in): [128, 2b*768]
    F1DT = BF16 if cfg["f1bc_bf16"] else FP32
    f1r_d = nc.declare_dram_parameter("f1r", [128, B * NS], F1DT, isOutput=False)
    # consts: [bias, bias-1, 0, 0]
    consts_d = nc.declare_dram_parameter("consts", [1, 4], FP32, isOutput=False)
    # transposed output [B, O, NS]
    out_d = nc.declare_dram_parameter("out", [B, O, NS], FP32, isOutput=True)

    NCH = 3                      # accumulator column chunks (2 per psum bank)
    CW = NS // NCH               # 256

    with tile.TileContext(nc) as tc:
        with (
            tc.tile_pool(name="const", bufs=1) as cpool,
            tc.tile_pool(name="sT", bufs=cfg["bufs_sT"]) as p_sT,
            tc.tile_pool(name="fts8", bufs=cfg["bufs_f8"]) as p_f8,
            tc.tile_pool(name="dt", bufs=cfg["bufs_d"]) as p_d,
            tc.tile_pool(name="tct", bufs=cfg["bufs_tc"]) as p_tc,
            tc.tile_pool(name="fin", bufs=14) as p_fin,
            tc.tile_pool(name="finw", bufs=1) as p_finw,
        ):
            # ---------------- constants / setup ----------------
            wtg = cpool.tile([128, B, 2, O], BF16)
            f1bc2 = cpool.tile([128, B * NS], F1DT)
            f2c = cpool.tile([128, B, N // 128], FP32)
            consts = cpool.tile([1, 4], FP32)

            def load_consts():
                if cfg["dma_order"] == 0:
                    nc.sync.dma_start(f2c[:], f2c_d[:])
                    nc.sync.dma_start(f1bc2[:], f1r_d[:])
                    nc.sync.dma_start(
                        wtg[:], wtg_d.ap().rearrange("b p c o -> p b c o"))
                    nc.sync.dma_start(consts[:], consts_d[:])
                elif cfg["dma_order"] == 3:
                    # f1bc on the ACT HWDGE queue in parallel with SP's f2c
                    nc.scalar.dma_start(f1bc2[:], f1r_d[:])
                    nc.sync.dma_start(f2c[:], f2c_d[:])
                    nc.sync.dma_start(
                        wtg[:], wtg_d.ap().rearrange("b p c o -> p b c o"))
                    nc.sync.dma_start(consts[:], consts_d[:])
                elif cfg["dma_order"] == 1:
                    nc.sync.dma_start(f1bc2[:], f1r_d[:])
                    nc.sync.dma_start(f2c[:], f2c_d[:])
                    nc.sync.dma_start(
                        wtg[:], wtg_d.ap().rearrange("b p c o -> p b c o"))
                    nc.sync.dma_start(consts[:], consts_d[:])
                else:
                    # order 2: consts emitted AFTER load0 (see loop); here only
                    # the fast ones first
                    nc.sync.dma_start(
                        wtg[:], wtg_d.ap().rearrange("b p c o -> p b c o"))
                    nc.sync.dma_start(f2c[:], f2c_d[:])
                    nc.sync.dma_start(f1bc2[:], f1r_d[:])
                    nc.sync.dma_start(consts[:], consts_d[:])

            constbc = cpool.tile([128, 4], FP32)
            ones_sb = cpool.tile([128, B, 2, 16], SBD)
            nc.gpsimd.memset(ones_sb[:], 1.0)
            one_f = cpool.tile([1, 1], FP32)
            nc.gpsimd.memset(one_f[:], 1.0)
            onesbc = cpool.tile([128, CW], FP32)
            nc.gpsimd.memset(onesbc[:], 1.0)

            with (
                tc.tile_pool(name="psV", bufs=1, space="PSUM") as psV,
                tc.tile_pool(name="psF", bufs=cfg["bufs_fppD"], space="PSUM") as psF,
                tc.tile_pool(name="psS", bufs=1, space="PSUM") as psS,
            ):
                # valsT packed [128o, 6q, 256i]: q = b*3+c, two q per psum bank.
                # start=True (whole-bank clear) only on even q at p==0.
                valsTq = psV.tile([128, 2 * NCH, CW], FP32, name="vT", tag="vT")
                s_acc = psS.tile([1, B, O], FP32, name="sacc", tag="sacc")

                f8_tiles = {}
                tc_tiles = {}

                def stage_load(k):
                    # seqT slice [2b, 2hc, 128h, 512n] -> sT [128h, 2b, 2hc, 512n]
                    sT = p_sT.tile([128, B, 2, 512], BF16, name="sT", tag="sT")
                    if k == 0 and cfg["split_load0"]:
                        # two half-loads so proj(0, pp=0) starts sooner
                        for h in range(2):
                            nc.sync.dma_start(
                                sT[:, :, :, h * 256:(h + 1) * 256],
                                seqT_d[:, :, :, h * 256:(h + 1) * 256]
                                .rearrange("b c h n -> h b c n"),
                            )
                    else:
                        nc.sync.dma_start(
                            sT[:],
                            seqT_d[:, :, :, k * 512:(k + 1) * 512].rearrange(
                                "b c h n -> h b c n"
                            ),
                        )
                    return sT

                def proj_mm(k, sT):
                    # fppD [128n, 2pp, 2b, 2sub, 128o] accumulated over hc;
                    # each pp-slice is exactly one psum bank.
                    fppD = psF.tile([128, 2, B, 2, O], FP32, name="fppD", tag="fppD")
                    for pp in range(2):
                        first = True
                        for b in range(B):
                            for sub in range(2):
                                nsl = slice((2 * pp + sub) * 128,
                                            (2 * pp + sub + 1) * 128)
                                for hc in range(2):
                                    nc.tensor.matmul(
                                        fppD[:, pp, b, sub],
                                        lhsT=sT[:, b, hc, nsl],
                                        rhs=wtg[:, b, hc],
                                        start=first, stop=(hc == 1),
                                        skip_group_check=True,
                                    )
                                    first = False
                    return fppD

                def proj_copy(k, fppD):
                    # psum -> sbuf cast (one op per block), mostly on ACT
                    f8 = p_f8.tile([128, 2, B, 2, O], SBD, name="f8", tag="f8")
                    if k in DVE_COPY_BLOCKS:
                        nc.vector.tensor_copy(f8[:], fppD[:])
                    else:
                        nc.scalar.activation(f8[:], fppD[:], AF.Copy)
                    # colsum accumulation (independent of scores; closes early)
                    for pp in range(2):
                        for b in range(B):
                            if dr8:
                                nc.tensor.matmul(
                                    s_acc[:, b], lhsT=ones_sb[:, b, :, 0:1],
                                    rhs=f8[:, pp, b],
                                    start=(k == 0 and pp == 0 and b == 0),
                                    stop=(k == NBLK - 1 and pp == 1),
                                    perf_mode=DR, skip_group_check=True,
                                )
                            else:
                                for sub in range(2):
                                    nc.tensor.matmul(
                                        s_acc[:, b], lhsT=ones_sb[:, b, sub, 0:1],
                                        rhs=f8[:, pp, b, sub],
                                        start=(k == 0 and pp == 0 and b == 0
                                               and sub == 0),
                                        stop=(k == NBLK - 1 and pp == 1
                                              and sub == 1),
                                        skip_group_check=True,
                                    )
                    f8_tiles[k] = f8

                tc_last = cpool.tile([128, 4, NS], SBD)

                def stage_scores(k):
                    # d [128j, 4jt, 768i] fp32; one custom-dve op per j-tile,
                    # one tanh for the whole block. The LAST block's scores are
                    # hoisted early (into a dedicated tc tile) so the tail only
                    # waits on its projection, not the whole d/tanh chain.
                    jt0 = 4 * k
                    d = p_d.tile([128, 4, NS], FP32, name="d", tag="d")
                    for sub in range(4):
                        nc.vector._custom_dve(
                            diff_lrelu,
                            out=d[:, sub],
                            in0=f1bc2[:, 0:NS],
                            in1=f1bc2[:, NS:2 * NS],
                            s0=f2c[:, 0, jt0 + sub:jt0 + sub + 1],
                            s1=f2c[:, 1, jt0 + sub:jt0 + sub + 1],
                            imm2=0.01,
                        )
                    if k == NBLK - 1:
                        t = tc_last
                    else:
                        t = p_tc.tile([128, 4, NS], SBD, name="tc", tag="tc")
                    nc.scalar.activation(t[:], d[:], AF.Tanh)
                    tc_tiles[k] = (t, 0)

                def stage_b(k):
                    f8 = f8_tiles.pop(k)
                    t, toff = tc_tiles.pop(k)
                    # block NBLK-1 (hoisted scores) is EMITTED before NBLK-2,
                    # so the accumulation stop rides block NBLK-2
                    last = k == (NBLK - 2 if cfg["swap_last_b"] else NBLK - 1)
                    # chunk-major so the last block closes accumulators one
                    # chunk at a time (finalize starts during remaining DRs).
                    # start=True clears a whole psum bank, so only the FIRST
                    # q emitted into each bank (emission order q0,q3,q1,q4,q2,q5
                    # -> banks {0:q0, 1:q3, 2:q4}) may issue it.
                    START_Q = {0, 3, 4}
                    for c in range(NCH):
                        for b in range(B):
                            q = b * NCH + c
                            for pp in range(2):
                                if dr8:
                                    nc.tensor.matmul(
                                        valsTq[:, q],
                                        lhsT=f8[:, pp, b],
                                        rhs=t[:, toff + 2 * pp:toff + 2 * pp + 2,
                                              c * CW:(c + 1) * CW],
                                        start=(k == 0 and pp == 0
                                               and q in START_Q),
                                        stop=(last and pp == 1),
                                        perf_mode=DR, skip_group_check=True,
                                    )
                                else:
                                    for sub in range(2):
                                        nc.tensor.matmul(
                                            valsTq[:, q],
                                            lhsT=f8[:, pp, b, sub],
                                            rhs=t[:, toff + 2 * pp + sub,
                                                  c * CW:(c + 1) * CW],
                                            start=(k == 0 and pp == 0 and sub == 0
                                                   and q in START_Q),
                                            stop=(last and pp == 1 and sub == 1),
                                            skip_group_check=True,
                                        )

                # ---------------- finalize ----------------
                # scol prep happens as soon as s_acc closes (last copy, well
                # before the last DR). y[b] = sgn[b]*P[b] + scol[b];
                # out = relu(y) - 1 + exp(min(y,0)); sgn: +1 (b=0), -1 (b=1).
                scol = cpool.tile([128, B, 3], FP32)   # [:, b, {s, s-1, -s}]

                def fin_prep():
                    # s_acc[0] = Sh0, s_acc[1] = -Sh1 (negated weights);
                    # scol[b] = sgn_acc*s_acc[b] + bias = Sh[b] + bias for b=0
                    # and  -(-Sh1) ... we need scol1 = Sh1 + bias, and the
                    # accumulator already holds -P1, so y1 = valsT1 + scol1.
                    s_sb = p_fin.tile([1, B, O], FP32, tag="fin_ssb")
                    nc.vector.tensor_copy(s_sb[:], s_acc[:])
                    for b in range(B):
                        colp = psF.tile([128, 1], FP32, name="colp", tag="fppD")
                        nc.tensor.matmul(
                            colp[:], lhsT=s_sb[0:1, b], rhs=one_f[:],
                            start=True, stop=True,
                        )
                        sgn = 1.0 if b == 0 else -1.0
                        # scol = sgn*colp + bias ; sm1 = scol - 1
                        nc.vector.tensor_scalar(
                            scol[:, b, 0:1], colp[:], sgn, constbc[:, 0:1],
                            ALU.mult, ALU.add
                        )
                        nc.vector.tensor_scalar(
                            scol[:, b, 1:2], scol[:, b, 0:1], -1.0, None, ALU.add
                        )
                        nc.vector.tensor_scalar(
                            scol[:, b, 2:3], scol[:, b, 0:1], -1.0, None, ALU.mult
                        )

                def fin_chunk(c):
                    # uniform: y = P' + s; out = max(y-1,-1) + exp(min(y,0))
                    # units with index >= 6 - fin_act_units use the ACT path
                    # (ry/nm/e) to take load off DVE's post-loop stream.
                    for b in range(B):
                        q = b * NCH + c
                        src = valsTq[:, q]
                        unit = c * B + b
                        e = p_fin.tile([128, CW], FP32, tag="fin_e")
                        o = p_fin.tile([128, CW], FP32, tag="fin_o")
                        if cfg["fin_nm_act"] and unit % 2 == 1:
                            # balanced split: DVE does rym, ACT does nm+e
                            rym = p_fin.tile([128, CW], FP32, tag="fin_rym")
                            nc.vector.tensor_scalar(
                                rym[:], src, scol[:, b, 1:2], -1.0,
                                ALU.add, ALU.max)
                            nm = p_fin.tile([128, CW], FP32, tag="fin_nm")
                            nc.scalar.activation(
                                nm[:], src, AF.Relu, bias=scol[:, b, 2:3],
                                scale=-1.0)
                            nc.scalar.activation(e[:], nm[:], AF.Exp, scale=-1.0)
                            nc.gpsimd.tensor_tensor(o[:], rym[:], e[:], ALU.add)
                            nc.sync.dma_start(
                                out_d[b, :, c * CW:(c + 1) * CW], o[:])
                            continue
                        if unit >= 2 * NCH - cfg["fin_act_units"]:
                            ry = p_fin.tile([128, CW], FP32, tag="fin_ry")
                            nc.scalar.activation(
                                ry[:], src, AF.Relu, bias=scol[:, b, 0:1])
                            nm = p_fin.tile([128, CW], FP32, tag="fin_nm")
                            nc.scalar.activation(
                                nm[:], src, AF.Relu, bias=scol[:, b, 2:3],
                                scale=-1.0)
                            nc.scalar.activation(e[:], nm[:], AF.Exp, scale=-1.0)
                            t1 = p_fin.tile([128, CW], FP32, tag="fin_t1")
                            nc.gpsimd.tensor_tensor(t1[:], ry[:], e[:], ALU.add)
                            nc.vector.tensor_scalar(o[:], t1[:], -1.0, None,
                                                    ALU.add)
                        else:
                            rym = p_fin.tile([128, CW], FP32, tag="fin_rym")
                            nc.vector.tensor_scalar(
                                rym[:], src, scol[:, b, 1:2], -1.0,
                                ALU.add, ALU.max)
                            m = p_fin.tile([128, CW], FP32, tag="fin_m")
                            nc.vector.tensor_scalar(
                                m[:], src, scol[:, b, 0:1], 0.0,
                                ALU.add, ALU.min)
                            nc.scalar.activation(e[:], m[:], AF.Exp)
                            nc.gpsimd.tensor_tensor(o[:], rym[:], e[:], ALU.add)
                        nc.sync.dma_start(out_d[b, :, c * CW:(c + 1) * CW], o[:])

                def fin_store():
                    pass

                # ---- software pipeline over blocks ----
                # PE p-state warmup: dummy matmuls into s_acc (cleared by the
                # real S-group's start=True) keep PE continuously busy from
                # ~t=0.5us so the first projections run at full clock.
                if cfg["pe_warmup"]:
                    ob = onesbc[:].bitcast(mybir.dt.float32r)
                    for _ in range(cfg["pe_warmup"]):
                        nc.tensor.matmul(
                            s_acc[:, 0], lhsT=ob[:, 0:1], rhs=ob[:, 0:O],
                            start=True, stop=True, skip_group_check=True,
                        )
                lag = max(1, min(cfg["lag"], NBLK))
                sT_tiles = {}
                fppD_tiles = {}
                if cfg["dma_order"] == 2:
                    sT_tiles[0] = stage_load(0)
                    load_consts()
                else:
                    load_consts()
                    sT_tiles[0] = stage_load(0)
                nc.gpsimd.partition_broadcast(constbc[:], consts[:])
                for k in range(NBLK + 1 + lag):
                    if 0 < k < NBLK:
                        sT_tiles[k] = stage_load(k)
                    j = k - 1
                    if 0 <= j < NBLK:
                        fppD_tiles[j] = proj_mm(j, sT_tiles.pop(j))
                        if j in DVE_COPY_BLOCKS:
                            # keep DVE's always-ready d-ops ahead of its copy
                            if j != NBLK - 1:
                                stage_scores(j)
                            proj_copy(j, fppD_tiles.pop(j))
                        else:
                            proj_copy(j, fppD_tiles.pop(j))
                            if j != NBLK - 1:
                                stage_scores(j)
                        if j == 1:
                            stage_scores(NBLK - 1)
                    if k == NBLK:
                        fin_prep()
                    if k >= 1 + lag:
                        j2 = k - 1 - lag
                        if cfg["swap_last_b"] and j2 >= NBLK - 2:
                            j2 = (2 * NBLK - 3) - j2   # 10<->11
                        stage_b(j2)
                if cfg["fin_wide"] == 3:
                    # 4 units of [128, 384]: fewer DVE per-op overheads; psum
                    # reads may span q-chunk bank boundaries (non-matmul ok)
                    HW = NS // 2
                    for b in range(B):
                        flat = valsTq[:, 3 * b:3 * b + 3].rearrange(
                            "p a b -> p (a b)")
                        for h in range(2):
                            srcv = flat[:, h * HW:(h + 1) * HW]
                            m = p_fin.tile([128, HW], FP32, tag="fin_hm")
                            nc.vector.tensor_scalar(
                                m[:], srcv, scol[:, b, 0:1], 0.0,
                                ALU.add, ALU.min)
                            rym = p_fin.tile([128, HW], FP32, tag="fin_hrym")
                            nc.vector.tensor_scalar(
                                rym[:], srcv, scol[:, b, 1:2], -1.0,
                                ALU.add, ALU.max)
                            e = p_fin.tile([128, HW], FP32, tag="fin_he")
                            nc.scalar.activation(e[:], m[:], AF.Exp)
                            o = p_fin.tile([128, HW], FP32, tag="fin_ho")
                            if b == 1 and h == 1:
                                nc.vector.tensor_tensor(o[:], rym[:], e[:],
                                                        ALU.add)
                            else:
                                nc.gpsimd.tensor_tensor(o[:], rym[:], e[:],
                                                        ALU.add)
                            nc.sync.dma_start(
                                out_d[b, :, h * HW:(h + 1) * HW], o[:])
                elif cfg["fin_wide"]:
                    # b's chunks are contiguous in valsTq: one [128,768] view
                    os_ = []
                    for b in range(B):
                        srcv = valsTq[:, 3 * b:3 * b + 3]
                        rym = p_finw.tile([128, NS], FP32, tag="fin_wrym")
                        nc.vector.tensor_scalar(
                            rym[:], srcv, scol[:, b, 1:2], -1.0,
                            ALU.add, ALU.max)
                        m = p_finw.tile([128, NS], FP32, tag="fin_wm")
                        nc.vector.tensor_scalar(
                            m[:], srcv, scol[:, b, 0:1], 0.0,
                            ALU.add, ALU.min)
                        e = p_finw.tile([128, NS], FP32, tag="fin_we")
                        nc.scalar.activation(e[:], m[:], AF.Exp)
                        o = p_finw.tile([128, NS], FP32, tag="fin_wo")
                        if b == 0:
                            nc.gpsimd.tensor_tensor(o[:], rym[:], e[:], ALU.add)
                        else:
                            nc.vector.tensor_tensor(o[:], rym[:], e[:], ALU.add)
                        nc.sync.dma_start(out_d[b], o[:])
                else:
                    for c in range(NCH):
                        fin_chunk(c)
                    fin_store()

    nc.compile()
    return nc


def make_in_maps(seq, W_fts, f1_w, f1_b, f2_w, f2_b, bias):
    seq = np.asarray(seq, dtype=np.float32)
    W = np.asarray(W_fts, dtype=np.float32)
    f1_w = np.asarray(f1_w, dtype=np.float32).reshape(-1)
    f2_w = np.asarray(f2_w, dtype=np.float32).reshape(-1)
    f1_bs = float(np.asarray(f1_b).reshape(-1)[0])
    f2_bs = float(np.asarray(f2_b).reshape(-1)[0])
    bs = float(np.asarray(bias).reshape(-1)[0])

    WT = np.ascontiguousarray(W.T)                      # [H, O]
    g1 = WT @ f1_w                                      # [H]
    g2 = WT @ f2_w

    # seqT [2b, 2hc, 128h, N] bf16
    seqT = np.ascontiguousarray(
        seq.transpose(0, 2, 1).reshape(B, 2, 128, N)
    ).astype(ml_dtypes.bfloat16)
    # wtg [2b, 128h, 2hc, 128o] bf16 = +/-0.5*WT (b=1 negated)
    wtg_half = (0.5 * WT).reshape(2, 128, O).transpose(1, 0, 2)   # [128, 2, O]
    wtg = np.ascontiguousarray(
        np.stack([wtg_half, -wtg_half], axis=0)
    ).astype(ml_dtypes.bfloat16)
    # f2h[b, j] = 0.5 * seq[b] @ g2  (fp32, no bias — folded into f1h)
    f2h = 0.5 * np.einsum("bnh,h->bn", seq, g2)         # [B, N]
    f2c = np.ascontiguousarray(
        f2h.reshape(B, N // 128, 128).transpose(2, 0, 1)
    ).astype(np.float32)                                # [128, B, 48]
    # f1h[b, i] = 0.5 * (seq[b] @ g1 + f1_b + f2_b), own shard per core
    f1h = 0.5 * (np.einsum("bnh,h->bn", seq, g1) + f1_bs + f2_bs)  # [B, N]
    consts = np.array([[bs, bs - 1.0, 0.0, 0.0]], np.float32)

    in_maps = []
    for c in range(NCORES):
        f1dt = ml_dtypes.bfloat16 if _F1BC_BF16[0] else np.float32
        f1r = np.ascontiguousarray(np.broadcast_to(
            f1h[:, c * NS:(c + 1) * NS].reshape(1, B * NS), (128, B * NS)
        )).astype(f1dt)
        in_maps.append({
            "seqT": seqT,
            "wtg": wtg,
            "f2c": f2c,
            "f1r": f1r,
            "consts": consts,
        })
    return in_maps


_NC_CACHE = []
_F1BC_BF16 = [DEFAULT_CFG["f1bc_bf16"]]


def kernel(seq, W_fts, f1_w, f1_b, f2_w, f2_b, bias):
    if not _NC_CACHE:
        _NC_CACHE.append(build_nc())
    nc = _NC_CACHE[0]
    in_maps = make_in_maps(seq, W_fts, f1_w, f1_b, f2_w, f2_b, bias)
    res = run_bass_kernel_spmd(nc, in_maps, core_ids=list(range(NCORES)))
    # per-core out is [B, O, NS] (transposed); gather + host-transpose
    outT = np.concatenate(
        [res.results[c]["out"] for c in range(NCORES)], axis=2
    )                                                    # [B, O, N]
    return np.ascontiguousarray(outT.transpose(0, 2, 1))


# revision 9
# speedup vs baseline: 1.0308x; 1.0062x over previous
"""Trainium2 Bass kernel v2 for the GAT-style attention head (B=2, N=6144, H=256, O=128).

Math (matching the reference):
  seq_fts = seq @ W_fts.T                       [B, N, O]
  f1 = seq_fts @ f1_w + f1_b                    [B, N]
  f2 = seq_fts @ f2_w + f2_b                    [B, N]
  z[b, j, i]  = leaky_relu(f1[b, i] + f2[b, j], 0.01)
  coefs[b,j,i] = softmax_b(z)   (B=2: c0 = sigmoid(z0 - z1), c1 = 1 - c0)
  vals[b, i, o] = sum_j coefs[b, j, i] * seq_fts[b, j, o]
  out = elu(vals + bias)

Key reformulation: c0 - 0.5 = 0.5*tanh((z0-z1)/2), and leaky_relu is
positively homogeneous, so with HALVED f1/f2 (0.5 folded into the host-packed
weights) the device computes
  dh[j,i] = lrelu(f1h[i]+f2h[j])|b0 - lrelu(...)|b1     (fused custom DVE op)
  tc = tanh(dh)                                          (ACT, fp8 out)
  P[b]  = sum_j tc[j,i] * fts_half[b,j,o]                (fp8 DoubleRow matmuls,
                                                          transposed acc [o, i])
  vals[0] = Sh[0] + P[0],  vals[1] = Sh[1] - P[1]        (Sh = colsum fts_half)
  out = elu(vals + bias) = relu(y) - 1 + exp(min(y, 0))

Sharding: each core owns 768 output rows i; seq streams fully through every
core (j loop). Host prep: seq pre-transposed/cast to bf16 [2b,2hc,128h,N],
f1h (own shard) / f2h (all j) computed on host fp32, W^T*0.5 packed bf16.
Output written transposed [B, O, NS] and fixed up on the host.

Schedule: 12 blocks of 2 j-tile-pairs (512 j rows each). Steady state is
DVE-bound (4 fused lrelu-diff ops per block); psum->sbuf fts casts ride
mostly on ACT (cfg dve_copy_every), tanh is one ACT op per block.
"""

import numpy as np
import ml_dtypes

import concourse.bacc as bacc
import concourse.bass as bass
import concourse.mybir as mybir
import concourse.tile as tile
from concourse.bass_utils import run_bass_kernel_spmd

B, N, H, O = 2, 6144, 256, 128
NCORES = 8
NS = N // NCORES          # 768 i-rows per core
NBLK = N // 512           # 12 blocks, each 2 pairs = 4 j-tiles = 512 j rows
FP32 = mybir.dt.float32
BF16 = mybir.dt.bfloat16
FP8 = mybir.dt.float8e4
AF = mybir.ActivationFunctionType
ALU = mybir.AluOpType
DR = mybir.MatmulPerfMode.DoubleRow

_DVE_OP_NAME = "DIFF_LRELU_ANT"

DEFAULT_CFG = dict(
    lag=1,                # stage-B lags scores by this many blocks
    bufs_sT=3,
    bufs_fppD=2,
    bufs_f8=3,
    bufs_d=3,
    bufs_tc=3,
    stageb="dr8",         # dr8 | bf16
    dve_copy_blocks=(9, 10, 11),  # blocks whose fts cast runs on DVE (rest ACT)
    split_tanh_block=-1,  # block whose tanh is emitted per-subtile
    store_queues=1,
    fin_act_units=0,
    pe_warmup=16,
    fin_wide=3,
    split_load0=True,
    swap_last_b=True,
    fin_nm_act=False,
    f1bc_bf16=True,
    dma_order=1,
    fin_dve_units=3,      # finalize units (of 6) using the DVE ry/m path
)


def _get_diff_lrelu_op():
    """Register (once) and return the fused custom DVE op:
    out = lrelu(in0 + s0) - lrelu(in1 + s1), slope imm2."""
    import concourse.dve_ops as dve_ops
    from concourse.dve_ops import OPS, DveOp

    for op in OPS:
        if op.name == _DVE_OP_NAME:
            return op

    from concourse.dve_spec import C0, C1, C2, Spec, Src0, Src1, lower, maxx
    from concourse.dve_uop import DveOpSpec

    a = Src0 + C0
    b = Src1 + C1
    spec = Spec(
        body=maxx(a, a * C2) - maxx(b, b * C2),
        reference=lambda in0, in1, s0, s1, imm2: (
            np.maximum(in0 + s0, (in0 + s0) * imm2)
            - np.maximum(in1 + s1, (in1 + s1) * imm2)
        ).astype(np.float32),
    )
    row = dve_ops._CUSTOM_DVE_ROW_BASE + len(OPS)
    shas = {}
    for ver in ("v3",):
        uops = lower(spec, ver=ver)
        shas[ver] = DveOpSpec(
            name=_DVE_OP_NAME, opcode=row, uops=uops, rd1_en=True
        ).sha(ver)
    op = DveOp(_DVE_OP_NAME, spec, subdim=False, uops_sha=shas)
    OPS.append(op)
    dve_ops.CUSTOM_DVE_SPECS[_DVE_OP_NAME] = spec
    dve_ops._SUB_OPCODE_FOR_NAME[_DVE_OP_NAME] = row
    return op


def build_nc(cfg=None):
    cfg = {**DEFAULT_CFG, **(cfg or {})}
    diff_lrelu = _get_diff_lrelu_op()
    dr8 = cfg["stageb"] == "dr8"
    SBD = FP8 if dr8 else BF16     # stage-B operand dtype
    DVE_COPY_BLOCKS = set(cfg["dve_copy_blocks"])

    nc = bacc.Bacc("TRN2", target_bir_lowering=False, debug=False, num_devices=NCORES)

    # seqT: [2b, 2hc, 128h, N] bf16 (host-transposed)
    seqT_d = nc.declare_dram_parameter("seqT", [B, 2, 128, N], BF16, isOutput=False)
    # wtg: [2b, 128h, 2hc, 128o] bf16 = +/-0.5 * W^T chunks (b=1 negated so
    # the accumulator holds -P1 and finalize is uniform y = P' + s)
    wtg_d = nc.declare_dram_parameter("wtg", [B, 128, 2, O], BF16, isOutput=False)
    # f2h columns for all j: [128j, 2b, 48jt] fp32
    f2c_d = nc.declare_dram_parameter("f2c", [128, B, N // 128], FP32, isOutput=False)
    # f1h broadcast for own shard (bias folded in): [128, 2b*768]
    F1DT = BF16 if cfg["f1bc_bf16"] else FP32
    f1r_d = nc.declare_dram_parameter("f1r", [128, B * NS], F1DT, isOutput=False)
    # consts: [bias, bias-1, 0, 0]
    consts_d = nc.declare_dram_parameter("consts", [1, 4], FP32, isOutput=False)
    # transposed output [B, O, NS]
    out_d = nc.declare_dram_parameter("out", [B, O, NS], FP32, isOutput=True)

    NCH = 3                      # accumulator column chunks (2 per psum bank)
    CW = NS // NCH               # 256

    with tile.TileContext(nc) as tc:
        with (
            tc.tile_pool(name="const", bufs=1) as cpool,
            tc.tile_pool(name="sT", bufs=cfg["bufs_sT"]) as p_sT,
            tc.tile_pool(name="fts8", bufs=cfg["bufs_f8"]) as p_f8,
            tc.tile_pool(name="dt", bufs=cfg["bufs_d"]) as p_d,
            tc.tile_pool(name="tct", bufs=cfg["bufs_tc"]) as p_tc,
            tc.tile_pool(name="fin", bufs=14) as p_fin,
            tc.tile_pool(name="finw", bufs=1) as p_finw,
        ):
            # ---------------- constants / setup ----------------
            wtg = cpool.tile([128, B, 2, O], BF16)
            f1bc2 = cpool.tile([128, B * NS], F1DT)
            f2c = cpool.tile([128, B, N // 128], FP32)
            consts = cpool.tile([1, 4], FP32)

            def load_consts():
                if cfg["dma_order"] == 0:
                    nc.sync.dma_start(f2c[:], f2c_d[:])
                    nc.sync.dma_start(f1bc2[:], f1r_d[:])
                    nc.sync.dma_start(
                        wtg[:], wtg_d.ap().rearrange("b p c o -> p b c o"))
                    nc.sync.dma_start(consts[:], consts_d[:])
                elif cfg["dma_order"] == 3:
                    # f1bc on the ACT HWDGE queue in parallel with SP's f2c
                    nc.scalar.dma_start(f1bc2[:], f1r_d[:])
                    nc.sync.dma_start(f2c[:], f2c_d[:])
                    nc.sync.dma_start(
                        wtg[:], wtg_d.ap().rearrange("b p c o -> p b c o"))
                    nc.sync.dma_start(consts[:], consts_d[:])
                elif cfg["dma_order"] == 1:
                    nc.sync.dma_start(f1bc2[:], f1r_d[:])
                    nc.sync.dma_start(f2c[:], f2c_d[:])
                    nc.sync.dma_start(
                        wtg[:], wtg_d.ap().rearrange("b p c o -> p b c o"))
                    nc.sync.dma_start(consts[:], consts_d[:])
                else:
                    # order 2: consts emitted AFTER load0 (see loop); here only
                    # the fast ones first
                    nc.sync.dma_start(
                        wtg[:], wtg_d.ap().rearrange("b p c o -> p b c o"))
                    nc.sync.dma_start(f2c[:], f2c_d[:])
                    nc.sync.dma_start(f1bc2[:], f1r_d[:])
                    nc.sync.dma_start(consts[:], consts_d[:])

            constbc = cpool.tile([128, 4], FP32)
            ones_sb = cpool.tile([128, B, 2, 16], SBD)
            nc.gpsimd.memset(ones_sb[:], 1.0)
            one_f = cpool.tile([1, 1], FP32)
            nc.gpsimd.memset(one_f[:], 1.0)
            onesbc = cpool.tile([128, CW], FP32)
            nc.gpsimd.memset(onesbc[:], 1.0)

            with (
                tc.tile_pool(name="psV", bufs=1, space="PSUM") as psV,
                tc.tile_pool(name="psF", bufs=cfg["bufs_fppD"], space="PSUM") as psF,
                tc.tile_pool(name="psS", bufs=1, space="PSUM") as psS,
            ):
                # valsT packed [128o, 6q, 256i]: q = b*3+c, two q per psum bank.
                # start=True (whole-bank clear) only on even q at p==0.
                valsTq = psV.tile([128, 2 * NCH, CW], FP32, name="vT", tag="vT")
                s_acc = psS.tile([1, B, O], FP32, name="sacc", tag="sacc")

                f8_tiles = {}
                tc_tiles = {}

                def stage_load(k):
                    # seqT slice [2b, 2hc, 128h, 512n] -> sT [128h, 2b, 2hc, 512n]
                    sT = p_sT.tile([128, B, 2, 512], BF16, name="sT", tag="sT")
                    if k == 0 and cfg["split_load0"]:
                        # two half-loads so proj(0, pp=0) starts sooner
                        for h in range(2):
                            nc.sync.dma_start(
                                sT[:, :, :, h * 256:(h + 1) * 256],
                                seqT_d[:, :, :, h * 256:(h + 1) * 256]
                                .rearrange("b c h n -> h b c n"),
                            )
                    else:
                        nc.sync.dma_start(
                            sT[:],
                            seqT_d[:, :, :, k * 512:(k + 1) * 512].rearrange(
                                "b c h n -> h b c n"
                            ),
                        )
                    return sT

                def proj_mm(k, sT):
                    # fppD [128n, 2pp, 2b, 2sub, 128o] accumulated over hc;
                    # each pp-slice is exactly one psum bank.
                    fppD = psF.tile([128, 2, B, 2, O], FP32, name="fppD", tag="fppD")
                    for pp in range(2):
                        first = True
                        for b in range(B):
                            for sub in range(2):
                                nsl = slice((2 * pp + sub) * 128,
                                            (2 * pp + sub + 1) * 128)
                                for hc in range(2):
                                    nc.tensor.matmul(
                                        fppD[:, pp, b, sub],
                                        lhsT=sT[:, b, hc, nsl],
                                        rhs=wtg[:, b, hc],
                                        start=first, stop=(hc == 1),
                                        skip_group_check=True,
                                    )
                                    first = False
                    return fppD

                def proj_copy(k, fppD):
                    # psum -> sbuf cast (one op per block), mostly on ACT
                    f8 = p_f8.tile([128, 2, B, 2, O], SBD, name="f8", tag="f8")
                    if k in DVE_COPY_BLOCKS:
                        nc.vector.tensor_copy(f8[:], fppD[:])
                    else:
                        nc.scalar.activation(f8[:], fppD[:], AF.Copy)
                    # colsum accumulation (independent of scores; closes early)
                    for pp in range(2):
                        for b in range(B):
                            if dr8:
                                nc.tensor.matmul(
                                    s_acc[:, b], lhsT=ones_sb[:, b, :, 0:1],
                                    rhs=f8[:, pp, b],
                                    start=(k == 0 and pp == 0 and b == 0),
                                    stop=(k == NBLK - 1 and pp == 1),
                                    perf_mode=DR, skip_group_check=True,
                                )
                            else:
                                for sub in range(2):
                                    nc.tensor.matmul(
                                        s_acc[:, b], lhsT=ones_sb[:, b, sub, 0:1],
                                        rhs=f8[:, pp, b, sub],
                                        start=(k == 0 and pp == 0 and b == 0
                                               and sub == 0),
                                        stop=(k == NBLK - 1 and pp == 1
                                              and sub == 1),
                                        skip_group_check=True,
                                    )
                    f8_tiles[k] = f8

                tc_last = cpool.tile([128, 4, NS], SBD)

                def stage_scores(k):
                    # d [128j, 4jt, 768i] fp32; one custom-dve op per j-tile,
                    # one tanh for the whole block. The LAST block's scores are
                    # hoisted early (into a dedicated tc tile) so the tail only
                    # waits on its projection, not the whole d/tanh chain.
                    jt0 = 4 * k
                    d = p_d.tile([128, 4, NS], FP32, name="d", tag="d")
                    for sub in range(4):
                        nc.vector._custom_dve(
                            diff_lrelu,
                            out=d[:, sub],
                            in0=f1bc2[:, 0:NS],
                            in1=f1bc2[:, NS:2 * NS],
                            s0=f2c[:, 0, jt0 + sub:jt0 + sub + 1],
                            s1=f2c[:, 1, jt0 + sub:jt0 + sub + 1],
                            imm2=0.01,
                        )
                    if k == NBLK - 1:
                        t = tc_last
                    else:
                        t = p_tc.tile([128, 4, NS], SBD, name="tc", tag="tc")
                    nc.scalar.activation(t[:], d[:], AF.Tanh)
                    tc_tiles[k] = (t, 0)

                def stage_b(k):
                    f8 = f8_tiles.pop(k)
                    t, toff = tc_tiles.pop(k)
                    # block NBLK-1 (hoisted scores) is EMITTED before NBLK-2,
                    # so the accumulation stop rides block NBLK-2
                    last = k == (NBLK - 2 if cfg["swap_last_b"] else NBLK - 1)
                    # chunk-major so the last block closes accumulators one
                    # chunk at a time (finalize starts during remaining DRs).
                    # start=True clears a whole psum bank, so only the FIRST
                    # q emitted into each bank (emission order q0,q3,q1,q4,q2,q5
                    # -> banks {0:q0, 1:q3, 2:q4}) may issue it.
                    START_Q = {0, 3, 4}
                    for c in range(NCH):
                        for b in range(B):
                            q = b * NCH + c
                            for pp in range(2):
                                if dr8:
                                    nc.tensor.matmul(
                                        valsTq[:, q],
                                        lhsT=f8[:, pp, b],
                                        rhs=t[:, toff + 2 * pp:toff + 2 * pp + 2,
                                              c * CW:(c + 1) * CW],
                                        start=(k == 0 and pp == 0
                                               and q in START_Q),
                                        stop=(last and pp == 1),
                                        perf_mode=DR, skip_group_check=True,
                                    )
                                else:
                                    for sub in range(2):
                                        nc.tensor.matmul(
                                            valsTq[:, q],
                                            lhsT=f8[:, pp, b, sub],
                                            rhs=t[:, toff + 2 * pp + sub,
                                                  c * CW:(c + 1) * CW],
                                            start=(k == 0 and pp == 0 and sub == 0
                                                   and q in START_Q),
                                            stop=(last and pp == 1 and sub == 1),
                                            skip_group_check=True,
                                        )

                # ---------------- finalize ----------------
                # scol prep happens as soon as s_acc closes (last copy, well
                # before the last DR). y[b] = sgn[b]*P[b] + scol[b];
                # out = relu(y) - 1 + exp(min(y,0)); sgn: +1 (b=0), -1 (b=1).
                scol = cpool.tile([128, B, 3], FP32)   # [:, b, {s, s-1, -s}]

                def fin_prep():
                    # s_acc[0] = Sh0, s_acc[1] = -Sh1 (negated weights);
                    # scol[b] = sgn_acc*s_acc[b] + bias = Sh[b] + bias for b=0
                    # and  -(-Sh1) ... we need scol1 = Sh1 + bias, and the
                    # accumulator already holds -P1, so y1 = valsT1 + scol1.
                    s_sb = p_fin.tile([1, B, O], FP32, tag="fin_ssb")
                    nc.vector.tensor_copy(s_sb[:], s_acc[:])
                    for b in range(B):
                        colp = psF.tile([128, 1], FP32, name="colp", tag="fppD")
                        nc.tensor.matmul(
                            colp[:], lhsT=s_sb[0:1, b], rhs=one_f[:],
                            start=True, stop=True,
                        )
                        sgn = 1.0 if b == 0 else -1.0
                        # scol = sgn*colp + bias ; sm1 = scol - 1
                        nc.vector.tensor_scalar(
                            scol[:, b, 0:1], colp[:], sgn, constbc[:, 0:1],
                            ALU.mult, ALU.add
                        )
                        nc.vector.tensor_scalar(
                            scol[:, b, 1:2], scol[:, b, 0:1], -1.0, None, ALU.add
                        )
                        nc.vector.tensor_scalar(
                            scol[:, b, 2:3], scol[:, b, 0:1], -1.0, None, ALU.mult
                        )

                def fin_chunk(c):
                    # uniform: y = P' + s; out = max(y-1,-1) + exp(min(y,0))
                    # units with index >= 6 - fin_act_units use the ACT path
                    # (ry/nm/e) to take load off DVE's post-loop stream.
                    for b in range(B):
                        q = b * NCH + c
                        src = valsTq[:, q]
                        unit = c * B + b
                        e = p_fin.tile([128, CW], FP32, tag="fin_e")
                        o = p_fin.tile([128, CW], FP32, tag="fin_o")
                        if cfg["fin_nm_act"] and unit % 2 == 1:
                            # balanced split: DVE does rym, ACT does nm+e
                            rym = p_fin.tile([128, CW], FP32, tag="fin_rym")
                            nc.vector.tensor_scalar(
                                rym[:], src, scol[:, b, 1:2], -1.0,
                                ALU.add, ALU.max)
                            nm = p_fin.tile([128, CW], FP32, tag="fin_nm")
                            nc.scalar.activation(
                                nm[:], src, AF.Relu, bias=scol[:, b, 2:3],
                                scale=-1.0)
                            nc.scalar.activation(e[:], nm[:], AF.Exp, scale=-1.0)
                            nc.gpsimd.tensor_tensor(o[:], rym[:], e[:], ALU.add)
                            nc.sync.dma_start(
                                out_d[b, :, c * CW:(c + 1) * CW], o[:])
                            continue
                        if unit >= 2 * NCH - cfg["fin_act_units"]:
                            ry = p_fin.tile([128, CW], FP32, tag="fin_ry")
                            nc.scalar.activation(
                                ry[:], src, AF.Relu, bias=scol[:, b, 0:1])
                            nm = p_fin.tile([128, CW], FP32, tag="fin_nm")
                            nc.scalar.activation(
                                nm[:], src, AF.Relu, bias=scol[:, b, 2:3],
                                scale=-1.0)
                            nc.scalar.activation(e[:], nm[:], AF.Exp, scale=-1.0)
                            t1 = p_fin.tile([128, CW], FP32, tag="fin_t1")
                            nc.gpsimd.tensor_tensor(t1[:], ry[:], e[:], ALU.add)
                            nc.vector.tensor_scalar(o[:], t1[:], -1.0, None,
                                                    ALU.add)
                        else:
                            rym = p_fin.tile([128, CW], FP32, tag="fin_rym")
                            nc.vector.tensor_scalar(
                                rym[:], src, scol[:, b, 1:2], -1.0,
                                ALU.add, ALU.max)
                            m = p_fin.tile([128, CW], FP32, tag="fin_m")
                            nc.vector.tensor_scalar(
                                m[:], src, scol[:, b, 0:1], 0.0,
                                ALU.add, ALU.min)
                            nc.scalar.activation(e[:], m[:], AF.Exp)
                            nc.gpsimd.tensor_tensor(o[:], rym[:], e[:], ALU.add)
                        nc.sync.dma_start(out_d[b, :, c * CW:(c + 1) * CW], o[:])

                def fin_store():
                    pass

                # ---- software pipeline over blocks ----
                # PE p-state warmup: dummy matmuls into s_acc (cleared by the
                # real S-group's start=True) keep PE continuously busy from
                # ~t=0.5us so the first projections run at full clock.
                if cfg["pe_warmup"]:
                    ob = onesbc[:].bitcast(mybir.dt.float32r)
                    for _ in range(cfg["pe_warmup"]):
                        nc.tensor.matmul(
                            s_acc[:, 0], lhsT=ob[:, 0:1], rhs=ob[:, 0:O],
                            start=True, stop=True, skip_group_check=True,
                        )
                lag = max(1, min(cfg["lag"], NBLK))
                sT_tiles = {}
                fppD_tiles = {}
                if cfg["dma_order"] == 2:
                    sT_tiles[0] = stage_load(0)
                    load_consts()
                else:
                    load_consts()
                    sT_tiles[0] = stage_load(0)
                nc.gpsimd.partition_broadcast(constbc[:], consts[:])
                for k in range(NBLK + 1 + lag):
                    if 0 < k < NBLK:
                        sT_tiles[k] = stage_load(k)
                    j = k - 1
                    if 0 <= j < NBLK:
                        fppD_tiles[j] = proj_mm(j, sT_tiles.pop(j))
                        if j in DVE_COPY_BLOCKS:
                            # keep DVE's always-ready d-ops ahead of its copy
                            if j != NBLK - 1:
                                stage_scores(j)
                            proj_copy(j, fppD_tiles.pop(j))
                        else:
                            proj_copy(j, fppD_tiles.pop(j))
                            if j != NBLK - 1:
                                stage_scores(j)
                        if j == 1:
                            stage_scores(NBLK - 1)
                    if k == NBLK:
                        fin_prep()
                    if k >= 1 + lag:
                        j2 = k - 1 - lag
                        if cfg["swap_last_b"] and j2 >= NBLK - 2:
                            j2 = (2 * NBLK - 3) - j2   # 10<->11
                        stage_b(j2)
                if cfg["fin_wide"] == 3:
                    # 4 units of [128, 384]: fewer DVE per-op overheads; psum
                    # reads may span q-chunk bank boundaries (non-matmul ok)
                    HW = NS // 2
                    for b in range(B):
                        flat = valsTq[:, 3 * b:3 * b + 3].rearrange(
                            "p a b -> p (a b)")
                        for h in range(2):
                            srcv = flat[:, h * HW:(h + 1) * HW]
                            m = p_fin.tile([128, HW], FP32, tag="fin_hm")
                            nc.vector.tensor_scalar(
                                m[:], srcv, scol[:, b, 0:1], 0.0,
                                ALU.add, ALU.min)
                            rym = p_fin.tile([128, HW], FP32, tag="fin_hrym")
                            nc.vector.tensor_scalar(
                                rym[:], srcv, scol[:, b, 1:2], -1.0,
                                ALU.add, ALU.max)
                            e = p_fin.tile([128, HW], FP32, tag="fin_he")
                            nc.scalar.activation(e[:], m[:], AF.Exp)
                            o = p_fin.tile([128, HW], FP32, tag="fin_ho")
                            if b == 1 and h == 1:
                                nc.vector.tensor_tensor(o[:], rym[:], e[:],
                                                        ALU.add)
                            else:
                                nc.gpsimd.tensor_tensor(o[:], rym[:], e[:],
                                                        ALU.add)
                            nc.sync.dma_start(
                                out_d[b, :, h * HW:(h + 1) * HW], o[:])
                elif cfg["fin_wide"]:
                    # b's chunks are contiguous in valsTq: one [128,768] view
                    os_ = []
                    for b in range(B):
                        srcv = valsTq[:, 3 * b:3 * b + 3]
                        rym = p_finw.tile([128, NS], FP32, tag="fin_wrym")
                        nc.vector.tensor_scalar(
                            rym[:], srcv, scol[:, b, 1:2], -1.0,
                            ALU.add, ALU.max)
                        m = p_finw.tile([128, NS], FP32, tag="fin_wm")
                        nc.vector.tensor_scalar(
                            m[:], srcv, scol[:, b, 0:1], 0.0,
                            ALU.add, ALU.min)
                        e = p_finw.tile([128, NS], FP32, tag="fin_we")
                        nc.scalar.activation(e[:], m[:], AF.Exp)
                        o = p_finw.tile([128, NS], FP32, tag="fin_wo")
                        if b == 0:
                            nc.gpsimd.tensor_tensor(o[:], rym[:], e[:], ALU.add)
                        else:
                            nc.vector.tensor_tensor(o[:], rym[:], e[:], ALU.add)
                        nc.sync.dma_start(out_d[b], o[:])
                else:
                    for c in range(NCH):
                        fin_chunk(c)
                    fin_store()

    nc.compile()
    return nc


def make_in_maps(seq, W_fts, f1_w, f1_b, f2_w, f2_b, bias):
    seq = np.asarray(seq, dtype=np.float32)
    W = np.asarray(W_fts, dtype=np.float32)
    f1_w = np.asarray(f1_w, dtype=np.float32).reshape(-1)
    f2_w = np.asarray(f2_w, dtype=np.float32).reshape(-1)
    f1_bs = float(np.asarray(f1_b).reshape(-1)[0])
    f2_bs = float(np.asarray(f2_b).reshape(-1)[0])
    bs = float(np.asarray(bias).reshape(-1)[0])

    WT = np.ascontiguousarray(W.T)                      # [H, O]
    g1 = WT @ f1_w                                      # [H]
    g2 = WT @ f2_w

    # seqT [2b, 2hc, 128h, N] bf16
    seqT = np.ascontiguousarray(
        seq.transpose(0, 2, 1).reshape(B, 2, 128, N)
    ).astype(ml_dtypes.bfloat16)
    # wtg [2b, 128h, 2hc, 128o] bf16 = +/-0.5*WT (b=1 negated)
    wtg_half = (0.5 * WT).reshape(2, 128, O).transpose(1, 0, 2)   # [128, 2, O]
    wtg = np.ascontiguousarray(
        np.stack([wtg_half, -wtg_half], axis=0)
    ).astype(ml_dtypes.bfloat16)
    # f2h[b, j] = 0.5 * seq[b] @ g2  (fp32, no bias — folded into f1h)
    f2h = 0.5 * np.einsum("bnh,h->bn", seq, g2)         # [B, N]
    f2c = np.ascontiguousarray(
        f2h.reshape(B, N // 128, 128).transpose(2, 0, 1)
    ).astype(np.float32)                                # [128, B, 48]
    # f1h[b, i] = 0.5 * (seq[b] @ g1 + f1_b + f2_b), own shard per core
    f1h = 0.5 * (np.einsum("bnh,h->bn", seq, g1) + f1_bs + f2_bs)  # [B, N]
    consts = np.array([[bs, bs - 1.0, 0.0, 0.0]], np.float32)

    in_maps = []
    for c in range(NCORES):
        f1dt = ml_dtypes.bfloat16 if _F1BC_BF16[0] else np.float32
        f1r = np.ascontiguousarray(np.broadcast_to(
            f1h[:, c * NS:(c + 1) * NS].reshape(1, B * NS), (128, B * NS)
        )).astype(f1dt)
        in_maps.append({
            "seqT": seqT,
            "wtg": wtg,
            "f2c": f2c,
            "f1r": f1r,
            "consts": consts,
        })
    return in_maps


_NC_CACHE = []
_F1BC_BF16 = [DEFAULT_CFG["f1bc_bf16"]]


def kernel(seq, W_fts, f1_w, f1_b, f2_w, f2_b, bias):
    if not _NC_CACHE:
        _NC_CACHE.append(build_nc())
    nc = _NC_CACHE[0]
    in_maps = make_in_maps(seq, W_fts, f1_w, f1_b, f2_w, f2_b, bias)
    res = run_bass_kernel_spmd(nc, in_maps, core_ids=list(range(NCORES)))
    # per-core out is [B, O, NS] (transposed); gather + host-transpose
    outT = np.concatenate(
        [res.results[c]["out"] for c in range(NCORES)], axis=2
    )                                                    # [B, O, N]
    return np.ascontiguousarray(outT.transpose(0, 2, 1))
